# revision 37
# baseline (speedup 1.0000x reference)
"""Trainium2 Bass kernel for chunked local self-attention (8-core SPMD).

Model (hardcoded from the problem spec):
  B=2, S=8192, HID=1024, NH=16, DH=64, CHUNK=64, N_BEFORE=1, N_AFTER=0,
  decoder-causal, softmax over a 128-wide rolled window per 64-chunk.

Sharding: sequence-parallel over 8 cores. Core i handles seq rows
[1024*i, 1024*(i+1)) of both batches; the 64-row (1-chunk) front halo the
local attention window needs is ring-exchanged on device (wrapped,
matching jnp.roll semantics; the wrapped window is masked out exactly as
in the reference).

Wire-format optimizations (the end-to-end time is dominated by the axon
host<->device tunnel at ~30 MB/s, not device compute):
  - hidden_states are sent as per-row int8 (amax/127 per-row scale),
    dequantized to bf16 on device by the scalar engine; no halo
    duplication (on-device ReduceScatter ring exchange).
  - weights are sent as bf16 1/8-row-shards and AllGather'd on device
    (96 MB of replicated f32 -> 6 MB on the wire).
  - output returns as per-row-int8 + f32 row scales (minimizes both the
    donated zero-init upload and the result download), decoded on host.
  - masks/identity are generated on device (affine_select).

Per-core pipeline (per batch):
  1. DMA int8 X slab rows, ACT-dequant to bf16, PE-transpose to XT
     [hid, row] (bf16).
  2. QKV projections on PE in bf16:
       QT[outd, row], KT[outd, row] (K pre-scaled on host),
       V[row, outd] (+ones col) via lhsT/rhs role swaps of XT.
  3. Attention per (512-row subpanel, head-pair): banded matmuls per
     128-row V tile rt:
       PT_raw[kv, qi] = KT-tile x QT-span   (one MM per tile, kv on psum
                                             partitions; both heads of a
                                             pair run concurrently on
                                             disjoint PE row groups)
       PT = exp(PT_raw) * mask   (ACT exp psum->bf16, DVE mask multiply)
       OT[qi, d] += PT^T x [V|1] (single PSUM accumulator; row 64 gathers
                                  the softmax denominators)
       scale rows by 1/sums into an f32 assembly buffer; per-row amax
       int8 quantization (round-to-nearest via the 1.5*2^23 trick) and
       4+4 batched DMAs (int8 data + f32 row scales) per subpanel.
"""

import sys

sys.path.insert(0, "/opt/trn_rl_repo")

import numpy as np
import ml_dtypes
from concurrent.futures import ThreadPoolExecutor

B, S, HID = 2, 8192, 1024
NH, DH = 16, 64
CHUNK = 64
CORES = 8
SLICE = S // CORES          # 1024 q rows per core per batch
HALO = 128                  # 2-chunk front pad: 1 zero chunk + 1 ring-
                            # exchanged halo chunk (keeps tiles 128-aligned)
SLAB = SLICE + HALO         # 1152 rows of XT/KT per core per batch
NRT = SLAB // 128           # 9 row tiles of V / X
NSP = SLICE // 512          # 2 attention subpanels per batch
KS = 384                    # KT projection free-dim span
WSH = HID // CORES          # 128 weight rows per core shard

_CACHE = {}


def _build():
    import concourse.bass as bass
    import concourse.tile as tile
    from concourse.tile import add_dep_helper
    from concourse import mybir, bacc

    F32 = mybir.dt.float32
    BF16 = mybir.dt.bfloat16
    I8 = mybir.dt.int8
    EXP = mybir.ActivationFunctionType.Exp
    COPY = mybir.ActivationFunctionType.Copy

    nc = bacc.Bacc("TRN2", target_bir_lowering=False, debug=False,
                   num_devices=CORES)

    xq = nc.dram_tensor("xq", [B, SLICE, HID], I8, kind="ExternalInput")
    xs = nc.dram_tensor("xs", [B, SLICE, 1], F32, kind="ExternalInput")
    wqs = nc.dram_tensor("wqs", [WSH, HID], BF16, kind="ExternalInput")
    wks = nc.dram_tensor("wks", [WSH, HID], BF16, kind="ExternalInput")
    wvs = nc.dram_tensor("wvs", [WSH, HID], BF16, kind="ExternalInput")
    out = nc.dram_tensor("out", [B, SLICE, HID], I8, kind="ExternalOutput")
    osc = nc.dram_tensor("osc", [B, SLICE, 1], F32, kind="ExternalOutput")

    # qi col spans (local to a 512-col subpanel) of the band MM for V-tile
    # l = rt - 4*sp, and the PV accumulation order/splits: (l, lo, hi) with
    # lo/hi in subpanel cols; pt-tile cols are [lo - SPANS[l][0], ...).
    SPANS = [(0, 64), (0, 192), (128, 320), (256, 448), (384, 512)]
    # PV accumulation: (qi block c4, V tile l, pt col lo, pt col hi); per
    # block the full-window tile (M=128) writes first, the half-window
    # (M=64) accumulates onto partitions [0:64). All 8 MMs form one ordered
    # psum group; stop is set on the last M=128 and the last MM so the
    # per-partition group flags clear for the whole bank.
    PV_O2 = [(0, 1, 0, 128), (0, 0, 0, 64),
             (1, 2, 0, 128), (1, 1, 128, 192),
             (2, 3, 0, 128), (2, 2, 128, 192),
             (3, 4, 0, 128), (3, 3, 128, 192)]
    # mask slice of mgen [128, 192] = [D0|D1|D2] per l (mgen is generated
    # on device below: block Dd = masks for chunk-distance d / d-1)
    MSLICE = [(128, 192), (0, 192), (0, 192), (0, 192), (0, 128)]

    with tile.TileContext(nc) as tc:
        with (
            tc.tile_pool(name="dram", bufs=1, space="DRAM") as dram,
            tc.tile_pool(name="big", bufs=1) as big,
            tc.tile_pool(name="xin", bufs=4) as xin_pool,
            tc.tile_pool(name="xsc", bufs=4) as xsc_pool,
            tc.tile_pool(name="wqk", bufs=4) as wqk_pool,
            tc.tile_pool(name="wvp", bufs=2) as wv_pool,
            tc.tile_pool(name="pt", bufs=34) as pt_pool,
            tc.tile_pool(name="oacc", bufs=1) as oacc_pool,
            tc.tile_pool(name="oq", bufs=4) as oq_pool,
            tc.tile_pool(name="rec", bufs=4) as rec_pool,
            tc.tile_pool(name="misc", bufs=1) as misc,
            tc.tile_pool(name="pss", bufs=4, space="PSUM") as ps_small,
            tc.tile_pool(name="psp", bufs=2, space="PSUM") as ps_proj,
            tc.tile_pool(name="pso", bufs=2, space="PSUM") as ps_o,
        ):
            # --- weight all-gather: 1/8 row shards -> full [HID, HID] ---
            wfull = []
            for name, wsh in (("wq", wqs), ("wk", wks), ("wv", wvs)):
                bounce = dram.tile([WSH, HID], BF16, tag=f"{name}b")
                full = dram.tile([HID, HID], BF16, tag=f"{name}f")
                nc.sync.dma_start(out=bounce[:], in_=wsh[:])
                nc.gpsimd.collective_compute(
                    "AllGather", mybir.AluOpType.bypass,
                    replica_groups=[list(range(CORES))],
                    ins=[bounce.opt()], outs=[full.opt()])
                wfull.append(full)
            wq_full, wk_full, wv_full = wfull

            # constants generated on device (saves wire bytes): identity for
            # PE transposes and the mask family mgen = [D0|D1|D2] where
            # block Dd holds the masks for chunk-distance d (top 64 rows)
            # and d-1 (bottom 64 rows): 0 -> causal, 1 -> ones, else zeros.
            ones_sb = misc.tile([128, 128], BF16, tag="ones")
            nc.vector.memset(ones_sb[:], 1.0)
            ident_sb = misc.tile([128, 128], BF16, tag="ident")
            nc.gpsimd.affine_select(
                ident_sb[:], ones_sb[:], pattern=[[1, 128]],
                compare_op=mybir.AluOpType.is_equal, fill=0.0,
                base=0, channel_multiplier=-1)
            mgen_sb = misc.tile([128, 192], BF16, tag="mgen")
            # D0: f - p >= 0 -> top causal, bottom (p>=64) all fill(0)
            nc.gpsimd.affine_select(
                mgen_sb[:, 0:64], ones_sb[:, 0:64], pattern=[[1, 64]],
                compare_op=mybir.AluOpType.is_ge, fill=0.0,
                base=0, channel_multiplier=-1)
            # D1: f - p + 64 >= 0 -> top all ones, bottom causal
            nc.gpsimd.affine_select(
                mgen_sb[:, 64:128], ones_sb[:, 0:64], pattern=[[1, 64]],
                compare_op=mybir.AluOpType.is_ge, fill=0.0,
                base=64, channel_multiplier=-1)
            # D2: p - 64 >= 0 -> top zeros, bottom all ones
            nc.gpsimd.affine_select(
                mgen_sb[:, 128:192], ones_sb[:, 0:64], pattern=[[0, 64]],
                compare_op=mybir.AluOpType.is_ge, fill=0.0,
                base=-64, channel_multiplier=1)
            # --- halo ring-exchange: each core sends its LAST 64 rows of
            # both batches (dequantized bf16, rows on partitions: batch b at
            # partitions [64b, 64b+64)) to the next core. SPMD-safe slot
            # selection: slot j of the RS input carries the halo iff
            # j == (pid+1) mod 8 (masked multiply by an is_equal of the
            # PE-broadcast partition id); ReduceScatter(add) then hands core
            # i slot i = core (i-1)'s halo. The 64 rows of the front chunk
            # beyond the halo are masked out by mgen/mfirst anyway and are
            # fed as zeros.
            rs_in = dram.tile([CORES, 128, HID], BF16, tag="rsin")
            rs_out = dram.tile([128, HID], BF16, tag="rsout")
            hq = misc.tile([128, HID], I8, tag="hq")
            hsc = misc.tile([128, 1], F32, tag="hsc")
            hbf = misc.tile([128, HID], BF16, tag="hbf")
            for b in range(B):
                nc.sync.dma_start(out=hq[64 * b:64 * b + 64, :],
                                  in_=xq[b, SLICE - 64:SLICE, :])
                nc.sync.dma_start(out=hsc[64 * b:64 * b + 64, :],
                                  in_=xs[b, SLICE - 64:SLICE, :])
            nc.scalar.activation(hbf[:], hq[:], COPY, scale=hsc[:])
            pid_u = misc.tile([1, 1], mybir.dt.uint32, tag="pidu")
            nc.sync.dma_start(out=pid_u[:],
                              in_=nc.partition_id_tensor[0:1, 0:1])
            pid_b = misc.tile([1, 1], BF16, tag="pidb")
            nc.vector.tensor_copy(pid_b[:], pid_u[:])
            pidp = ps_small.tile([128, 192], F32, tag="pp", name="pidp")
            nc.tensor.matmul(pidp[:, 0:1], ones_sb[0:1, :], pid_b[:],
                             start=True, stop=True)
            pidv = misc.tile([128, 1], F32, tag="pidv")
            nc.vector.tensor_scalar_add(pidv[:], pidp[:, 0:1], 1.0)
            # mfirst = D2 slice of mgen, zeroed on core 0 (no wrap-attend
            # for the first chunk of the sequence)
            selm = misc.tile([128, 1], BF16, tag="selm")
            nc.vector.tensor_scalar(selm[:], pidv[:], 1.5, None,
                                    op0=mybir.AluOpType.is_ge)
            mfirst_sb = misc.tile([128, 64], BF16, tag="mfirst")
            nc.vector.tensor_tensor(mfirst_sb[:], mgen_sb[:, 128:192],
                                    selm[:].to_broadcast((128, 64)),
                                    mybir.AluOpType.mult)
            for j in range(CORES):
                sel = rec_pool.tile([128, 1], BF16, tag="sel", name="sel")
                nc.vector.tensor_scalar(
                    sel[:], pidv[:], float(j) if j else 8.0, None,
                    op0=mybir.AluOpType.is_equal)
                slot = oq_pool.tile([128, HID], BF16, tag="slot",
                                    name="slot")
                nc.vector.tensor_tensor(
                    slot[:], hbf[:], sel[:].to_broadcast((128, HID)),
                    mybir.AluOpType.mult)
                nc.sync.dma_start(out=rs_in[j], in_=slot[:])
            nc.gpsimd.collective_compute(
                "ReduceScatter", mybir.AluOpType.add,
                replica_groups=[list(range(CORES))],
                ins=[rs_in.opt()], outs=[rs_out.opt()])

            for b in range(B):
                XT = big.tile([128, 8, SLAB], BF16, tag="xt")
                QT = big.tile([128, 8, SLICE], BF16, tag="qt")
                KT = big.tile([128, 8, SLAB], BF16, tag="kt")
                V1 = big.tile([128, NRT, NH, DH + 1], BF16, tag="v1")
                nc.vector.memset(V1[:, :, :, DH:DH + 1], 1.0)

                # --- Phase A: load int8 + dequant + transpose X ---
                # slab tile 0 = [64 zero rows | 64 ring-exchanged halo rows];
                # tiles 1..8 come from this core's own 1024 rows.
                for rt in range(NRT):
                    xin = xin_pool.tile([128, HID], BF16, tag="xin",
                                        name="xin")
                    if rt == 0:
                        nc.vector.memset(xin[0:64, :], 0.0)
                        nc.sync.dma_start(out=xin[64:128, :],
                                          in_=rs_out[64 * b:64 * b + 64, :])
                    else:
                        r0 = 128 * rt - 128
                        xin8 = xin_pool.tile([128, HID], I8, tag="xin8",
                                             name="xin8")
                        nc.sync.dma_start(out=xin8[:],
                                          in_=xq[b, r0:r0 + 128, :])
                        xsc = xsc_pool.tile([128, 1], F32, tag="xsc")
                        nc.sync.dma_start(out=xsc[:],
                                          in_=xs[b, r0:r0 + 128, :])
                        nc.scalar.activation(xin[:], xin8[:], COPY,
                                             scale=xsc[:])
                    for hp in range(4):
                        # transpose passes through lhsT dtype -> bf16 psum;
                        # full-bank alloc keeps the pool slot size uniform
                        tpf = ps_proj.tile([128, 1024], BF16, tag="proj",
                                           name="tp")
                        tp = tpf[:, 0:256]
                        tm1 = nc.tensor.matmul(
                            tp[:, 0:128], xin[:, 256 * hp:256 * hp + 128],
                            ident_sb[:], is_transpose=True,
                            start=True, stop=False)
                        tm2 = nc.tensor.matmul(
                            tp[:, 128:256],
                            xin[:, 256 * hp + 128:256 * hp + 256],
                            ident_sb[:], is_transpose=True,
                            start=False, stop=True)
                        add_dep_helper(tm2.ins, tm1.ins, sync=False,
                                       reason="psum group order")
                        nc.vector.tensor_copy(
                            XT[:, 2 * hp:2 * hp + 2,
                               128 * rt:128 * rt + 128], tp[:])

                # --- Phase B: projections ---
                # QT: lhsT = wq tile [hid, outd], rhs = XT -> [outd, row] bf16
                for ot in range(8):
                    wt = wqk_pool.tile([128, 8, 128], BF16, tag="wqk")
                    nc.sync.dma_start(
                        out=wt[:],
                        in_=wq_full[:, 128 * ot:128 * ot + 128].rearrange(
                            "(ht p) o -> p ht o", p=128))
                    for half in range(2):
                        qp = ps_proj.tile([128, 512], F32, tag="proj")
                        for ht in range(8):
                            nc.tensor.matmul(
                                qp[:], wt[:, ht, :],
                                XT[:, ht, HALO + 512 * half:
                                   HALO + 512 * half + 512],
                                start=(ht == 0), stop=(ht == 7))
                        nc.vector.tensor_copy(
                            QT[:, ot, 512 * half:512 * half + 512], qp[:])

                # KT: same, over all SLAB cols (K pre-scaled on host)
                for ot in range(8):
                    wt = wqk_pool.tile([128, 8, 128], BF16, tag="wqk")
                    nc.sync.dma_start(
                        out=wt[:],
                        in_=wk_full[:, 128 * ot:128 * ot + 128].rearrange(
                            "(ht p) o -> p ht o", p=128))
                    for ks in range(SLAB // KS):
                        kpf = ps_proj.tile([128, 512], F32, tag="proj",
                                           name="kpf")
                        kp = kpf[:, 0:KS]
                        for ht in range(8):
                            nc.tensor.matmul(
                                kp[:], wt[:, ht, :],
                                XT[:, ht, KS * ks:KS * ks + KS],
                                start=(ht == 0), stop=(ht == 7))
                        nc.vector.tensor_copy(
                            KT[:, ot, KS * ks:KS * ks + KS], kp[:])

                # V: lhsT = XT row tile, rhs = wv [hid, outd] -> [row, outd]
                for oh in range(2):
                    wvt = wv_pool.tile([128, 8, 512], BF16, tag="wv")
                    nc.sync.dma_start(
                        out=wvt[:],
                        in_=wv_full[:, 512 * oh:512 * oh + 512].rearrange(
                            "(ht p) o -> p ht o", p=128))
                    for rt in range(NRT):
                        vp = ps_proj.tile([128, 512], F32, tag="proj")
                        for ht in range(8):
                            nc.tensor.matmul(
                                vp[:], XT[:, ht, 128 * rt:128 * rt + 128],
                                wvt[:, ht, :], start=(ht == 0),
                                stop=(ht == 7))
                        nc.vector.tensor_copy(
                            V1[:, rt, 8 * oh:8 * oh + 8, 0:DH], vp[:])

                # --- Phase C: attention ---
                for sp in range(NSP):
                    oacc = oacc_pool.tile([128, 4, HID], F32, tag="oacc")

                    def emit_mm1s(sp, t):
                        pts = {}
                        for l in (1, 0, 2, 3, 4):
                            rt = 4 * sp + l
                            lo, hi = SPANS[l]
                            pps = []
                            for e in range(2):
                                pp = ps_small.tile([128, 192], F32,
                                                   tag="pp", name="pp")
                                nc.tensor.matmul(
                                    pp[:, 0:hi - lo],
                                    KT[64 * e:64 * e + 64, t,
                                       128 * rt:128 * rt + 128],
                                    QT[64 * e:64 * e + 64, t,
                                       512 * sp + lo:512 * sp + hi],
                                    start=True, stop=True,
                                    tile_position=(64 * e, 0))
                                pps.append(pp)
                            for e in range(2):
                                pt = pt_pool.tile([128, 192], BF16, tag="pt",
                                                  name="pt")
                                nc.scalar.activation(pt[:, 0:hi - lo],
                                                     pps[e][:, 0:hi - lo],
                                                     EXP)
                                if l == 0 and sp == 0:
                                    msk = mfirst_sb[:]
                                else:
                                    ml, mh = MSLICE[l]
                                    msk = mgen_sb[:, ml:mh]
                                nc.vector.tensor_tensor(
                                    pt[:, 0:hi - lo], pt[:, 0:hi - lo], msk,
                                    mybir.AluOpType.mult)
                                pts[(e, l)] = pt
                        return pts

                    def emit_pv(sp, t, pts):
                        for e in range(2):
                            h = 2 * t + e
                            # O[qi, d] directly: lhsT = PT slice (qi block on
                            # psum partitions), rhs = [V|1]; all 4 qi blocks
                            # share one psum bank; per block the full-window
                            # tile writes first, the half-window accumulates.
                            ops = ps_o.tile([128, 4, DH + 1], F32, tag="o",
                                            name="ops")
                            prev = None
                            for i, (c4, l, plo, phi) in enumerate(PV_O2):
                                rt = 4 * sp + l
                                mm = nc.tensor.matmul(
                                    ops[0:phi - plo, c4, :],
                                    pts[(e, l)][:, plo:phi],
                                    V1[:, rt, h, :],
                                    start=(i == 0),
                                    stop=(i >= len(PV_O2) - 2),
                                    skip_group_check=True)
                                if prev is not None:
                                    # keep the per-block psum groups in
                                    # program order (flag-clear before the
                                    # next group's start)
                                    add_dep_helper(mm.ins, prev.ins,
                                                   sync=False,
                                                   reason="psum group order")
                                prev = mm
                            rec = rec_pool.tile([128, 4], F32, tag="rec")
                            nc.vector.reciprocal(rec[:], ops[:, :, DH:DH + 1])
                            nc.vector.tensor_tensor(
                                oacc[:, :, DH * h:DH * h + DH],
                                ops[:, :, 0:DH],
                                rec[:, :, None].to_broadcast((128, 4, DH)),
                                mybir.AluOpType.mult)

                    pending = []
                    for t in range(NH // 2):
                        pts = emit_mm1s(sp, t)
                        pending.append((t, pts))
                        if len(pending) > 2:
                            pt_, pts_ = pending.pop(0)
                            emit_pv(sp, pt_, pts_)
                    for pt_, pts_ in pending:
                        emit_pv(sp, pt_, pts_)

                    # int8-quantize the subpanel per out row: amax over hid,
                    # scale = amax/126 (margin vs reciprocal rounding), then
                    # round-to-nearest via the +1.5*2^23 float trick.
                    RB = 12582912.0  # 1.5 * 2**23
                    qam = rec_pool.tile([128, 4], F32, tag="qam", name="qam")
                    nc.vector.tensor_reduce(qam[:], oacc[:],
                                            mybir.AxisListType.X,
                                            mybir.AluOpType.max,
                                            apply_absolute_value=True)
                    oscs = rec_pool.tile([128, 4], F32, tag="oscs",
                                         name="oscs")
                    nc.vector.tensor_scalar_mul(oscs[:], qam[:], 1.0 / 126.0)
                    qrec = rec_pool.tile([128, 4], F32, tag="qrec",
                                         name="qrec")
                    nc.vector.reciprocal(qrec[:], oscs[:])
                    for c4 in range(4):
                        r0 = 512 * sp + 128 * c4
                        t1 = oq_pool.tile([128, HID], F32, tag="t1",
                                          name="t1")
                        nc.vector.tensor_scalar(
                            t1[:], oacc[:, c4, :], qrec[:, c4:c4 + 1], RB,
                            op0=mybir.AluOpType.mult,
                            op1=mybir.AluOpType.add)
                        q8 = oq_pool.tile([128, HID], I8, tag="q8",
                                          name="q8")
                        nc.vector.tensor_scalar(
                            q8[:], t1[:], RB, None,
                            op0=mybir.AluOpType.subtract)
                        nc.sync.dma_start(out=out[b, r0:r0 + 128, :],
                                          in_=q8[:])
                        nc.sync.dma_start(out=osc[b, r0:r0 + 128, :],
                                          in_=oscs[:, c4:c4 + 1])
    nc.compile()
    return nc


def _inputs_for_core(i, xq8, xsc, wq, wk, wv):
    return {
        "xq": xq8[:, SLICE * i:SLICE * (i + 1)],
        "xs": xsc[:, SLICE * i:SLICE * (i + 1), None],
        "wqs": wq[WSH * i:WSH * (i + 1)],
        "wks": wk[WSH * i:WSH * (i + 1)],
        "wvs": wv[WSH * i:WSH * (i + 1)],
    }


def kernel(hidden_states, Wq, Wk, Wv, _trace=False):
    from concourse.bass_utils import run_bass_kernel_spmd

    hidden_states = np.asarray(hidden_states, dtype=np.float32)
    Wq = np.asarray(Wq, dtype=np.float32).astype(ml_dtypes.bfloat16)
    Wk = (np.asarray(Wk, dtype=np.float32)
          * np.float32(1.0 / np.sqrt(DH))).astype(ml_dtypes.bfloat16)
    Wv = np.asarray(Wv, dtype=np.float32).astype(ml_dtypes.bfloat16)

    # per-row symmetric int8 quantization of hidden_states; numpy releases
    # the GIL on large array ops so chunked threads give real speedup
    if "pool" not in _CACHE:
        _CACHE["pool"] = ThreadPoolExecutor(max_workers=8)
        _CACHE["qtmp"] = np.empty((B, S, HID), dtype=np.float32)
        _CACHE["q8"] = np.empty((B, S, HID), dtype=np.int8)
    pool = _CACHE["pool"]
    tmp, xq8 = _CACHE["qtmp"], _CACHE["q8"]
    xsc = np.empty((B, S), dtype=np.float32)

    def _quant_chunk(b, s0, s1):
        h = hidden_states[b, s0:s1]
        t = tmp[b, s0:s1]
        am = np.maximum(np.maximum(h.max(axis=-1), -h.min(axis=-1)),
                        np.float32(1e-20))
        xsc[b, s0:s1] = am * np.float32(1.0 / 127.0)
        np.multiply(h, (np.float32(127.0) / am)[:, None], out=t)
        np.rint(t, out=t)
        xq8[b, s0:s1] = t

    CH = S // 4
    list(pool.map(lambda a: _quant_chunk(*a),
                  [(b, c * CH, (c + 1) * CH) for b in range(B)
                   for c in range(4)]))

    if "nc" not in _CACHE:
        _CACHE["nc"] = _build()
    nc = _CACHE["nc"]

    in_maps = [_inputs_for_core(i, xq8, xsc, Wq, Wk, Wv)
               for i in range(CORES)]
    res = run_bass_kernel_spmd(nc, in_maps, list(range(CORES)), trace=_trace)
    _CACHE["last"] = res
    full = np.empty((B, S, HID), dtype=np.float32)

    def _decode(i):
        r = res.results[i]
        np.multiply(r["out"], r["osc"],
                    out=full[:, SLICE * i:SLICE * (i + 1), :])

    list(_CACHE["pool"].map(_decode, range(CORES)))
    return full


# revision 46
# speedup vs baseline: 1.0067x; 1.0067x over previous
"""Trainium2 Bass kernel for chunked local self-attention (8-core SPMD).

Model (hardcoded from the problem spec):
  B=2, S=8192, HID=1024, NH=16, DH=64, CHUNK=64, N_BEFORE=1, N_AFTER=0,
  decoder-causal, softmax over a 128-wide rolled window per 64-chunk.

Sharding: sequence-parallel over 8 cores. Core i handles seq rows
[1024*i, 1024*(i+1)) of both batches; the 64-row (1-chunk) front halo the
local attention window needs is ring-exchanged on device (wrapped,
matching jnp.roll semantics; the wrapped window is masked out exactly as
in the reference).

Wire-format optimizations (the end-to-end time is dominated by the axon
host<->device tunnel at ~30 MB/s, not device compute):
  - hidden_states are sent as per-row int8 (amax/127 per-row scale),
    dequantized to bf16 on device by the scalar engine; no halo
    duplication (on-device ReduceScatter ring exchange).
  - weights are sent as bf16 1/8-row-shards and AllGather'd on device
    (96 MB of replicated f32 -> 6 MB on the wire).
  - output returns as per-row-int8 + f32 row scales (minimizes both the
    donated zero-init upload and the result download), decoded on host.
  - masks/identity are generated on device (affine_select).

Per-core pipeline (per batch):
  1. DMA int8 X slab rows, ACT-dequant to bf16, PE-transpose to XT
     [hid, row] (bf16).
  2. QKV projections on PE in bf16:
       QT[outd, row], KT[outd, row] (K pre-scaled on host),
       V[row, outd] (+ones col) via lhsT/rhs role swaps of XT.
  3. Attention per (512-row subpanel, head-pair): banded matmuls per
     128-row V tile rt:
       PT_raw[kv, qi] = KT-tile x QT-span   (one MM per tile, kv on psum
                                             partitions; both heads of a
                                             pair run concurrently on
                                             disjoint PE row groups)
       PT = exp(PT_raw) * mask   (ACT exp psum->bf16, DVE mask multiply)
       OT[qi, d] += PT^T x [V|1] (single PSUM accumulator; row 64 gathers
                                  the softmax denominators)
       scale rows by 1/sums into an f32 assembly buffer; per-row amax
       int8 quantization (round-to-nearest via the 1.5*2^23 trick) and
       4+4 batched DMAs (int8 data + f32 row scales) per subpanel.
"""

import sys

sys.path.insert(0, "/opt/trn_rl_repo")

import numpy as np
import ml_dtypes
from concurrent.futures import ThreadPoolExecutor

B, S, HID = 2, 8192, 1024
NH, DH = 16, 64
CHUNK = 64
CORES = 8
SLICE = S // CORES          # 1024 q rows per core per batch
HALO = 128                  # 2-chunk front pad: 1 zero chunk + 1 ring-
                            # exchanged halo chunk (keeps tiles 128-aligned)
SLAB = SLICE + HALO         # 1152 rows of XT/KT per core per batch
NRT = SLAB // 128           # 9 row tiles of V / X
NSP = SLICE // 512          # 2 attention subpanels per batch
KS = 384                    # KT projection free-dim span
WSH = HID // CORES          # 128 weight rows per core shard

_CACHE = {}


def _build():
    import concourse.bass as bass
    import concourse.tile as tile
    from concourse.tile import add_dep_helper
    from concourse import mybir, bacc

    F32 = mybir.dt.float32
    BF16 = mybir.dt.bfloat16
    I8 = mybir.dt.int8
    EXP = mybir.ActivationFunctionType.Exp
    COPY = mybir.ActivationFunctionType.Copy

    nc = bacc.Bacc("TRN2", target_bir_lowering=False, debug=False,
                   num_devices=CORES)

    xq = nc.dram_tensor("xq", [B, SLICE, HID], I8, kind="ExternalInput")
    xs = nc.dram_tensor("xs", [B, SLICE, 1], F32, kind="ExternalInput")
    wqs = nc.dram_tensor("wqs", [WSH, HID], I8, kind="ExternalInput")
    wks = nc.dram_tensor("wks", [WSH, HID], I8, kind="ExternalInput")
    wvs = nc.dram_tensor("wvs", [WSH, HID], I8, kind="ExternalInput")
    wsc = nc.dram_tensor("wsc", [3, HID], F32, kind="ExternalInput")
    out = nc.dram_tensor("out", [B, SLICE, HID], I8, kind="ExternalOutput")
    osc = nc.dram_tensor("osc", [B, SLICE, 1], F32, kind="ExternalOutput")

    # qi col spans (local to a 512-col subpanel) of the band MM for V-tile
    # l = rt - 4*sp, and the PV accumulation order/splits: (l, lo, hi) with
    # lo/hi in subpanel cols; pt-tile cols are [lo - SPANS[l][0], ...).
    SPANS = [(0, 64), (0, 192), (128, 320), (256, 448), (384, 512)]
    # PV accumulation: (qi block c4, V tile l, pt col lo, pt col hi); per
    # block the full-window tile (M=128) writes first, the half-window
    # (M=64) accumulates onto partitions [0:64). All 8 MMs form one ordered
    # psum group; stop is set on the last M=128 and the last MM so the
    # per-partition group flags clear for the whole bank.
    PV_O2 = [(0, 1, 0, 128), (0, 0, 0, 64),
             (1, 2, 0, 128), (1, 1, 128, 192),
             (2, 3, 0, 128), (2, 2, 128, 192),
             (3, 4, 0, 128), (3, 3, 128, 192)]
    # mask slice of mgen [128, 192] = [D0|D1|D2] per l (mgen is generated
    # on device below: block Dd = masks for chunk-distance d / d-1)
    MSLICE = [(128, 192), (0, 192), (0, 192), (0, 192), (0, 128)]

    with tile.TileContext(nc) as tc:
        with (
            tc.tile_pool(name="dram", bufs=1, space="DRAM") as dram,
            tc.tile_pool(name="big", bufs=1) as big,
            tc.tile_pool(name="xin", bufs=4) as xin_pool,
            tc.tile_pool(name="xsc", bufs=4) as xsc_pool,
            tc.tile_pool(name="wqk", bufs=4) as wqk_pool,
            tc.tile_pool(name="wvp", bufs=2) as wv_pool,
            tc.tile_pool(name="pt", bufs=34) as pt_pool,
            tc.tile_pool(name="oacc", bufs=1) as oacc_pool,
            tc.tile_pool(name="oq", bufs=4) as oq_pool,
            tc.tile_pool(name="rec", bufs=4) as rec_pool,
            tc.tile_pool(name="misc", bufs=1) as misc,
            tc.tile_pool(name="pss", bufs=4, space="PSUM") as ps_small,
            tc.tile_pool(name="psp", bufs=2, space="PSUM") as ps_proj,
            tc.tile_pool(name="pso", bufs=2, space="PSUM") as ps_o,
        ):
            # --- weight all-gather: 1/8 int8 row shards -> full [HID, HID];
            # weights are per-hid-row int8 (amax/127), dequantized to bf16
            # at tile load with per-partition ACT scales (hid sits on
            # partitions in all three projection matmul layouts) ---
            wfull = []
            for name, wsh in (("wq", wqs), ("wk", wks), ("wv", wvs)):
                bounce = dram.tile([WSH, HID], I8, tag=f"{name}b")
                full = dram.tile([HID, HID], I8, tag=f"{name}f")
                nc.sync.dma_start(out=bounce[:], in_=wsh[:])
                nc.gpsimd.collective_compute(
                    "AllGather", mybir.AluOpType.bypass,
                    replica_groups=[list(range(CORES))],
                    ins=[bounce.opt()], outs=[full.opt()])
                wfull.append(full)
            wq_full, wk_full, wv_full = wfull
            wsc_sb = []
            for m in range(3):
                wss = misc.tile([128, 8], F32, tag=f"wsc{m}")
                nc.sync.dma_start(
                    out=wss[:],
                    in_=wsc[m, :].rearrange("(ht p) -> p ht", p=128))
                wsc_sb.append(wss)

            # constants generated on device (saves wire bytes): identity for
            # PE transposes and the mask family mgen = [D0|D1|D2] where
            # block Dd holds the masks for chunk-distance d (top 64 rows)
            # and d-1 (bottom 64 rows): 0 -> causal, 1 -> ones, else zeros.
            ones_sb = misc.tile([128, 128], BF16, tag="ones")
            nc.vector.memset(ones_sb[:], 1.0)
            ident_sb = misc.tile([128, 128], BF16, tag="ident")
            nc.gpsimd.affine_select(
                ident_sb[:], ones_sb[:], pattern=[[1, 128]],
                compare_op=mybir.AluOpType.is_equal, fill=0.0,
                base=0, channel_multiplier=-1)
            mgen_sb = misc.tile([128, 192], BF16, tag="mgen")
            # D0: f - p >= 0 -> top causal, bottom (p>=64) all fill(0)
            nc.gpsimd.affine_select(
                mgen_sb[:, 0:64], ones_sb[:, 0:64], pattern=[[1, 64]],
                compare_op=mybir.AluOpType.is_ge, fill=0.0,
                base=0, channel_multiplier=-1)
            # D1: f - p + 64 >= 0 -> top all ones, bottom causal
            nc.gpsimd.affine_select(
                mgen_sb[:, 64:128], ones_sb[:, 0:64], pattern=[[1, 64]],
                compare_op=mybir.AluOpType.is_ge, fill=0.0,
                base=64, channel_multiplier=-1)
            # D2: p - 64 >= 0 -> top zeros, bottom all ones
            nc.gpsimd.affine_select(
                mgen_sb[:, 128:192], ones_sb[:, 0:64], pattern=[[0, 64]],
                compare_op=mybir.AluOpType.is_ge, fill=0.0,
                base=-64, channel_multiplier=1)
            # --- halo ring-exchange: each core sends its LAST 64 rows of
            # both batches (dequantized bf16, rows on partitions: batch b at
            # partitions [64b, 64b+64)) to the next core. SPMD-safe slot
            # selection: slot j of the RS input carries the halo iff
            # j == (pid+1) mod 8 (masked multiply by an is_equal of the
            # PE-broadcast partition id); ReduceScatter(add) then hands core
            # i slot i = core (i-1)'s halo. The 64 rows of the front chunk
            # beyond the halo are masked out by mgen/mfirst anyway and are
            # fed as zeros.
            rs_in = dram.tile([CORES, 128, HID], BF16, tag="rsin")
            rs_out = dram.tile([128, HID], BF16, tag="rsout")
            hq = misc.tile([128, HID], I8, tag="hq")
            hsc = misc.tile([128, 1], F32, tag="hsc")
            hbf = misc.tile([128, HID], BF16, tag="hbf")
            for b in range(B):
                nc.sync.dma_start(out=hq[64 * b:64 * b + 64, :],
                                  in_=xq[b, SLICE - 64:SLICE, :])
                nc.sync.dma_start(out=hsc[64 * b:64 * b + 64, :],
                                  in_=xs[b, SLICE - 64:SLICE, :])
            nc.scalar.activation(hbf[:], hq[:], COPY, scale=hsc[:])
            pid_u = misc.tile([1, 1], mybir.dt.uint32, tag="pidu")
            nc.sync.dma_start(out=pid_u[:],
                              in_=nc.partition_id_tensor[0:1, 0:1])
            pid_b = misc.tile([1, 1], BF16, tag="pidb")
            nc.vector.tensor_copy(pid_b[:], pid_u[:])
            pidp = ps_small.tile([128, 192], F32, tag="pp", name="pidp")
            nc.tensor.matmul(pidp[:, 0:1], ones_sb[0:1, :], pid_b[:],
                             start=True, stop=True)
            pidv = misc.tile([128, 1], F32, tag="pidv")
            nc.vector.tensor_scalar_add(pidv[:], pidp[:, 0:1], 1.0)
            # mfirst = D2 slice of mgen, zeroed on core 0 (no wrap-attend
            # for the first chunk of the sequence)
            selm = misc.tile([128, 1], BF16, tag="selm")
            nc.vector.tensor_scalar(selm[:], pidv[:], 1.5, None,
                                    op0=mybir.AluOpType.is_ge)
            mfirst_sb = misc.tile([128, 64], BF16, tag="mfirst")
            nc.vector.tensor_tensor(mfirst_sb[:], mgen_sb[:, 128:192],
                                    selm[:].to_broadcast((128, 64)),
                                    mybir.AluOpType.mult)
            for j in range(CORES):
                sel = rec_pool.tile([128, 1], BF16, tag="sel", name="sel")
                nc.vector.tensor_scalar(
                    sel[:], pidv[:], float(j) if j else 8.0, None,
                    op0=mybir.AluOpType.is_equal)
                slot = oq_pool.tile([128, HID], BF16, tag="slot",
                                    name="slot")
                nc.vector.tensor_tensor(
                    slot[:], hbf[:], sel[:].to_broadcast((128, HID)),
                    mybir.AluOpType.mult)
                nc.sync.dma_start(out=rs_in[j], in_=slot[:])
            nc.gpsimd.collective_compute(
                "ReduceScatter", mybir.AluOpType.add,
                replica_groups=[list(range(CORES))],
                ins=[rs_in.opt()], outs=[rs_out.opt()])

            for b in range(B):
                XT = big.tile([128, 8, SLAB], BF16, tag="xt")
                QT = big.tile([128, 8, SLICE], BF16, tag="qt")
                KT = big.tile([128, 8, SLAB], BF16, tag="kt")
                V1 = big.tile([128, NRT, NH, DH + 1], BF16, tag="v1")
                nc.vector.memset(V1[:, :, :, DH:DH + 1], 1.0)

                # --- Phase A: load int8 + dequant + transpose X ---
                # slab tile 0 = [64 zero rows | 64 ring-exchanged halo rows];
                # tiles 1..8 come from this core's own 1024 rows.
                for rt in range(NRT):
                    xin = xin_pool.tile([128, HID], BF16, tag="xin",
                                        name="xin")
                    if rt == 0:
                        nc.vector.memset(xin[0:64, :], 0.0)
                        nc.sync.dma_start(out=xin[64:128, :],
                                          in_=rs_out[64 * b:64 * b + 64, :])
                    else:
                        r0 = 128 * rt - 128
                        xin8 = xin_pool.tile([128, HID], I8, tag="xin8",
                                             name="xin8")
                        nc.sync.dma_start(out=xin8[:],
                                          in_=xq[b, r0:r0 + 128, :])
                        xsc = xsc_pool.tile([128, 1], F32, tag="xsc")
                        nc.sync.dma_start(out=xsc[:],
                                          in_=xs[b, r0:r0 + 128, :])
                        nc.scalar.activation(xin[:], xin8[:], COPY,
                                             scale=xsc[:])
                    for hp in range(4):
                        # transpose passes through lhsT dtype -> bf16 psum;
                        # full-bank alloc keeps the pool slot size uniform
                        tpf = ps_proj.tile([128, 1024], BF16, tag="proj",
                                           name="tp")
                        tp = tpf[:, 0:256]
                        tm1 = nc.tensor.matmul(
                            tp[:, 0:128], xin[:, 256 * hp:256 * hp + 128],
                            ident_sb[:], is_transpose=True,
                            start=True, stop=False)
                        tm2 = nc.tensor.matmul(
                            tp[:, 128:256],
                            xin[:, 256 * hp + 128:256 * hp + 256],
                            ident_sb[:], is_transpose=True,
                            start=False, stop=True)
                        add_dep_helper(tm2.ins, tm1.ins, sync=False,
                                       reason="psum group order")
                        nc.vector.tensor_copy(
                            XT[:, 2 * hp:2 * hp + 2,
                               128 * rt:128 * rt + 128], tp[:])

                # --- Phase B: projections ---
                # QT: lhsT = wq tile [hid, outd], rhs = XT -> [outd, row] bf16
                for ot in range(8):
                    wt8 = wqk_pool.tile([128, 8, 128], I8, tag="wqk8",
                                        name="wt8")
                    nc.sync.dma_start(
                        out=wt8[:],
                        in_=wq_full[:, 128 * ot:128 * ot + 128].rearrange(
                            "(ht p) o -> p ht o", p=128))
                    wt = wqk_pool.tile([128, 8, 128], BF16, tag="wqk")
                    for ht in range(8):
                        nc.scalar.activation(wt[:, ht, :], wt8[:, ht, :],
                                             COPY,
                                             scale=wsc_sb[0][:, ht:ht + 1])
                    for half in range(2):
                        qp = ps_proj.tile([128, 512], F32, tag="proj")
                        for ht in range(8):
                            nc.tensor.matmul(
                                qp[:], wt[:, ht, :],
                                XT[:, ht, HALO + 512 * half:
                                   HALO + 512 * half + 512],
                                start=(ht == 0), stop=(ht == 7))
                        nc.vector.tensor_copy(
                            QT[:, ot, 512 * half:512 * half + 512], qp[:])

                # KT: same, over all SLAB cols (K pre-scaled on host)
                for ot in range(8):
                    wt8 = wqk_pool.tile([128, 8, 128], I8, tag="wqk8",
                                        name="wt8")
                    nc.sync.dma_start(
                        out=wt8[:],
                        in_=wk_full[:, 128 * ot:128 * ot + 128].rearrange(
                            "(ht p) o -> p ht o", p=128))
                    wt = wqk_pool.tile([128, 8, 128], BF16, tag="wqk")
                    for ht in range(8):
                        nc.scalar.activation(wt[:, ht, :], wt8[:, ht, :],
                                             COPY,
                                             scale=wsc_sb[1][:, ht:ht + 1])
                    for ks in range(SLAB // KS):
                        kpf = ps_proj.tile([128, 512], F32, tag="proj",
                                           name="kpf")
                        kp = kpf[:, 0:KS]
                        for ht in range(8):
                            nc.tensor.matmul(
                                kp[:], wt[:, ht, :],
                                XT[:, ht, KS * ks:KS * ks + KS],
                                start=(ht == 0), stop=(ht == 7))
                        nc.vector.tensor_copy(
                            KT[:, ot, KS * ks:KS * ks + KS], kp[:])

                # V: lhsT = XT row tile, rhs = wv [hid, outd] -> [row, outd]
                for oh in range(2):
                    wvt8 = wv_pool.tile([128, 8, 512], I8, tag="wv8",
                                        name="wvt8")
                    nc.sync.dma_start(
                        out=wvt8[:],
                        in_=wv_full[:, 512 * oh:512 * oh + 512].rearrange(
                            "(ht p) o -> p ht o", p=128))
                    wvt = wv_pool.tile([128, 8, 512], BF16, tag="wv")
                    for ht in range(8):
                        nc.scalar.activation(wvt[:, ht, :], wvt8[:, ht, :],
                                             COPY,
                                             scale=wsc_sb[2][:, ht:ht + 1])
                    for rt in range(NRT):
                        vp = ps_proj.tile([128, 512], F32, tag="proj")
                        for ht in range(8):
                            nc.tensor.matmul(
                                vp[:], XT[:, ht, 128 * rt:128 * rt + 128],
                                wvt[:, ht, :], start=(ht == 0),
                                stop=(ht == 7))
                        nc.vector.tensor_copy(
                            V1[:, rt, 8 * oh:8 * oh + 8, 0:DH], vp[:])

                # --- Phase C: attention ---
                for sp in range(NSP):
                    oacc = oacc_pool.tile([128, 4, HID], F32, tag="oacc")

                    def emit_mm1s(sp, t):
                        pts = {}
                        for l in (1, 0, 2, 3, 4):
                            rt = 4 * sp + l
                            lo, hi = SPANS[l]
                            pps = []
                            for e in range(2):
                                pp = ps_small.tile([128, 192], F32,
                                                   tag="pp", name="pp")
                                nc.tensor.matmul(
                                    pp[:, 0:hi - lo],
                                    KT[64 * e:64 * e + 64, t,
                                       128 * rt:128 * rt + 128],
                                    QT[64 * e:64 * e + 64, t,
                                       512 * sp + lo:512 * sp + hi],
                                    start=True, stop=True,
                                    tile_position=(64 * e, 0))
                                pps.append(pp)
                            for e in range(2):
                                pt = pt_pool.tile([128, 192], BF16, tag="pt",
                                                  name="pt")
                                nc.scalar.activation(pt[:, 0:hi - lo],
                                                     pps[e][:, 0:hi - lo],
                                                     EXP)
                                if l == 0 and sp == 0:
                                    msk = mfirst_sb[:]
                                else:
                                    ml, mh = MSLICE[l]
                                    msk = mgen_sb[:, ml:mh]
                                nc.vector.tensor_tensor(
                                    pt[:, 0:hi - lo], pt[:, 0:hi - lo], msk,
                                    mybir.AluOpType.mult)
                                pts[(e, l)] = pt
                        return pts

                    def emit_pv(sp, t, pts):
                        for e in range(2):
                            h = 2 * t + e
                            # O[qi, d] directly: lhsT = PT slice (qi block on
                            # psum partitions), rhs = [V|1]; all 4 qi blocks
                            # share one psum bank; per block the full-window
                            # tile writes first, the half-window accumulates.
                            ops = ps_o.tile([128, 4, DH + 1], F32, tag="o",
                                            name="ops")
                            prev = None
                            for i, (c4, l, plo, phi) in enumerate(PV_O2):
                                rt = 4 * sp + l
                                mm = nc.tensor.matmul(
                                    ops[0:phi - plo, c4, :],
                                    pts[(e, l)][:, plo:phi],
                                    V1[:, rt, h, :],
                                    start=(i == 0),
                                    stop=(i >= len(PV_O2) - 2),
                                    skip_group_check=True)
                                if prev is not None:
                                    # keep the per-block psum groups in
                                    # program order (flag-clear before the
                                    # next group's start)
                                    add_dep_helper(mm.ins, prev.ins,
                                                   sync=False,
                                                   reason="psum group order")
                                prev = mm
                            rec = rec_pool.tile([128, 4], F32, tag="rec")
                            nc.vector.reciprocal(rec[:], ops[:, :, DH:DH + 1])
                            nc.vector.tensor_tensor(
                                oacc[:, :, DH * h:DH * h + DH],
                                ops[:, :, 0:DH],
                                rec[:, :, None].to_broadcast((128, 4, DH)),
                                mybir.AluOpType.mult)

                    pending = []
                    for t in range(NH // 2):
                        pts = emit_mm1s(sp, t)
                        pending.append((t, pts))
                        if len(pending) > 2:
                            pt_, pts_ = pending.pop(0)
                            emit_pv(sp, pt_, pts_)
                    for pt_, pts_ in pending:
                        emit_pv(sp, pt_, pts_)

                    # int8-quantize the subpanel per out row: amax over hid,
                    # scale = amax/126 (margin vs reciprocal rounding), then
                    # round-to-nearest via the +1.5*2^23 float trick.
                    RB = 12582912.0  # 1.5 * 2**23
                    qam = rec_pool.tile([128, 4], F32, tag="qam", name="qam")
                    nc.vector.tensor_reduce(qam[:], oacc[:],
                                            mybir.AxisListType.X,
                                            mybir.AluOpType.max,
                                            apply_absolute_value=True)
                    oscs = rec_pool.tile([128, 4], F32, tag="oscs",
                                         name="oscs")
                    nc.vector.tensor_scalar_mul(oscs[:], qam[:], 1.0 / 126.0)
                    qrec = rec_pool.tile([128, 4], F32, tag="qrec",
                                         name="qrec")
                    nc.vector.reciprocal(qrec[:], oscs[:])
                    for c4 in range(4):
                        r0 = 512 * sp + 128 * c4
                        t1 = oq_pool.tile([128, HID], F32, tag="t1",
                                          name="t1")
                        nc.vector.tensor_scalar(
                            t1[:], oacc[:, c4, :], qrec[:, c4:c4 + 1], RB,
                            op0=mybir.AluOpType.mult,
                            op1=mybir.AluOpType.add)
                        q8 = oq_pool.tile([128, HID], I8, tag="q8",
                                          name="q8")
                        nc.vector.tensor_scalar(
                            q8[:], t1[:], RB, None,
                            op0=mybir.AluOpType.subtract)
                        nc.sync.dma_start(out=out[b, r0:r0 + 128, :],
                                          in_=q8[:])
                        nc.sync.dma_start(out=osc[b, r0:r0 + 128, :],
                                          in_=oscs[:, c4:c4 + 1])
    nc.compile()
    return nc


def _inputs_for_core(i, xq8, xsc, wq, wk, wv, wscales):
    return {
        "xq": xq8[:, SLICE * i:SLICE * (i + 1)],
        "xs": xsc[:, SLICE * i:SLICE * (i + 1), None],
        "wqs": wq[WSH * i:WSH * (i + 1)],
        "wks": wk[WSH * i:WSH * (i + 1)],
        "wvs": wv[WSH * i:WSH * (i + 1)],
        "wsc": wscales,
    }


def kernel(hidden_states, Wq, Wk, Wv, _trace=False):
    from concourse.bass_utils import run_bass_kernel_spmd

    hidden_states = np.asarray(hidden_states, dtype=np.float32)

    def _wquant(W, pre=1.0):
        W = np.asarray(W, dtype=np.float32) * np.float32(pre)
        am = np.maximum(np.abs(W).max(axis=1), np.float32(1e-20))  # [HID]
        sc = am * np.float32(1.0 / 127.0)
        q = np.rint(W * (np.float32(1.0) / sc)[:, None]).astype(np.int8)
        return q, sc

    Wq, sq = _wquant(Wq)
    Wk, sk = _wquant(Wk, pre=1.0 / np.sqrt(DH))
    Wv, sv = _wquant(Wv)
    wsc = np.stack([sq, sk, sv]).astype(np.float32)  # [3, HID]

    # per-row symmetric int8 quantization of hidden_states; numpy releases
    # the GIL on large array ops so chunked threads give real speedup
    if "pool" not in _CACHE:
        _CACHE["pool"] = ThreadPoolExecutor(max_workers=8)
        _CACHE["qtmp"] = np.empty((B, S, HID), dtype=np.float32)
        _CACHE["q8"] = np.empty((B, S, HID), dtype=np.int8)
    pool = _CACHE["pool"]
    tmp, xq8 = _CACHE["qtmp"], _CACHE["q8"]
    xsc = np.empty((B, S), dtype=np.float32)

    def _quant_chunk(b, s0, s1):
        h = hidden_states[b, s0:s1]
        t = tmp[b, s0:s1]
        am = np.maximum(np.maximum(h.max(axis=-1), -h.min(axis=-1)),
                        np.float32(1e-20))
        xsc[b, s0:s1] = am * np.float32(1.0 / 127.0)
        np.multiply(h, (np.float32(127.0) / am)[:, None], out=t)
        np.rint(t, out=t)
        xq8[b, s0:s1] = t

    CH = S // 4
    list(pool.map(lambda a: _quant_chunk(*a),
                  [(b, c * CH, (c + 1) * CH) for b in range(B)
                   for c in range(4)]))

    if "nc" not in _CACHE:
        _CACHE["nc"] = _build()
    nc = _CACHE["nc"]

    in_maps = [_inputs_for_core(i, xq8, xsc, Wq, Wk, Wv, wsc)
               for i in range(CORES)]
    res = run_bass_kernel_spmd(nc, in_maps, list(range(CORES)), trace=_trace)
    _CACHE["last"] = res
    full = np.empty((B, S, HID), dtype=np.float32)

    def _decode(i):
        r = res.results[i]
        np.multiply(r["out"], r["osc"],
                    out=full[:, SLICE * i:SLICE * (i + 1), :])

    list(_CACHE["pool"].map(_decode, range(CORES)))
    return full


# revision 48
# speedup vs baseline: 1.0181x; 1.0113x over previous
"""Trainium2 Bass kernel for chunked local self-attention (8-core SPMD).

Model (hardcoded from the problem spec):
  B=2, S=8192, HID=1024, NH=16, DH=64, CHUNK=64, N_BEFORE=1, N_AFTER=0,
  decoder-causal, softmax over a 128-wide rolled window per 64-chunk.

Sharding: sequence-parallel over 8 cores. Core i handles seq rows
[1024*i, 1024*(i+1)) of both batches; the 64-row (1-chunk) front halo the
local attention window needs is ring-exchanged on device (wrapped,
matching jnp.roll semantics; the wrapped window is masked out exactly as
in the reference).

Wire-format optimizations (the end-to-end time is dominated by the axon
host<->device tunnel at ~30 MB/s, not device compute):
  - hidden_states are sent as per-row int8 (amax/127 per-row scale),
    dequantized to bf16 on device by the scalar engine; no halo
    duplication (on-device ReduceScatter ring exchange).
  - weights are sent as per-hid-row int8 1/8-row-shards (+ f32 row
    scales) and AllGather'd on device, dequantized to bf16 at tile load
    with per-partition ACT scales (96 MB of replicated f32 -> 3.2 MB).
  - output returns as per-row-int8 + f32 row scales (minimizes both the
    donated zero-init upload and the result download), decoded on host.
  - masks/identity are generated on device (affine_select).

Per-core pipeline (per batch):
  1. DMA int8 X slab rows, ACT-dequant to bf16, PE-transpose to XT
     [hid, row] (bf16).
  2. QKV projections on PE in bf16:
       QT[outd, row], KT[outd, row] (K pre-scaled on host),
       V[row, outd] (+ones col) via lhsT/rhs role swaps of XT.
  3. Attention per (512-row subpanel, head-pair): banded matmuls per
     128-row V tile rt:
       PT_raw[kv, qi] = KT-tile x QT-span   (one MM per tile, kv on psum
                                             partitions; both heads of a
                                             pair run concurrently on
                                             disjoint PE row groups)
       PT = exp(PT_raw) * mask   (ACT exp psum->bf16, DVE mask multiply)
       OT[qi, d] += PT^T x [V|1] (single PSUM accumulator; row 64 gathers
                                  the softmax denominators)
       scale rows by 1/sums into an f32 assembly buffer; per-row amax
       int8 quantization (round-to-nearest via the 1.5*2^23 trick) and
       4+4 batched DMAs (int8 data + f32 row scales) per subpanel.
"""

import sys

sys.path.insert(0, "/opt/trn_rl_repo")

import numpy as np
from concurrent.futures import ThreadPoolExecutor

B, S, HID = 2, 8192, 1024
NH, DH = 16, 64
CHUNK = 64
CORES = 8
SLICE = S // CORES          # 1024 q rows per core per batch
HALO = 128                  # 2-chunk front pad: 1 zero chunk + 1 ring-
                            # exchanged halo chunk (keeps tiles 128-aligned)
SLAB = SLICE + HALO         # 1152 rows of XT/KT per core per batch
NRT = SLAB // 128           # 9 row tiles of V / X
NSP = SLICE // 512          # 2 attention subpanels per batch
KS = 384                    # KT projection free-dim span
WSH = HID // CORES          # 128 weight rows per core shard

_CACHE = {}


def _build():
    import concourse.bass as bass
    import concourse.tile as tile
    from concourse.tile import add_dep_helper
    from concourse import mybir, bacc

    F32 = mybir.dt.float32
    BF16 = mybir.dt.bfloat16
    I8 = mybir.dt.int8
    EXP = mybir.ActivationFunctionType.Exp
    COPY = mybir.ActivationFunctionType.Copy

    nc = bacc.Bacc("TRN2", target_bir_lowering=False, debug=False,
                   num_devices=CORES)

    xq = nc.dram_tensor("xq", [B, SLICE, HID], I8, kind="ExternalInput")
    xs = nc.dram_tensor("xs", [B, SLICE, 1], F32, kind="ExternalInput")
    wqs = nc.dram_tensor("wqs", [WSH, HID], I8, kind="ExternalInput")
    wks = nc.dram_tensor("wks", [WSH, HID], I8, kind="ExternalInput")
    wvs = nc.dram_tensor("wvs", [WSH, HID], I8, kind="ExternalInput")
    wsc = nc.dram_tensor("wsc", [3, HID], F32, kind="ExternalInput")
    out = nc.dram_tensor("out", [B, SLICE, HID], I8, kind="ExternalOutput")
    osc = nc.dram_tensor("osc", [B, SLICE, 1], F32, kind="ExternalOutput")

    # qi col spans (local to a 512-col subpanel) of the band MM for V-tile
    # l = rt - 4*sp, and the PV accumulation order/splits: (l, lo, hi) with
    # lo/hi in subpanel cols; pt-tile cols are [lo - SPANS[l][0], ...).
    SPANS = [(0, 64), (0, 192), (128, 320), (256, 448), (384, 512)]
    # PV accumulation: (qi block c4, V tile l, pt col lo, pt col hi); per
    # block the full-window tile (M=128) writes first, the half-window
    # (M=64) accumulates onto partitions [0:64). All 8 MMs form one ordered
    # psum group; stop is set on the last M=128 and the last MM so the
    # per-partition group flags clear for the whole bank.
    PV_O2 = [(0, 1, 0, 128), (0, 0, 0, 64),
             (1, 2, 0, 128), (1, 1, 128, 192),
             (2, 3, 0, 128), (2, 2, 128, 192),
             (3, 4, 0, 128), (3, 3, 128, 192)]
    # mask slice of mgen [128, 192] = [D0|D1|D2] per l (mgen is generated
    # on device below: block Dd = masks for chunk-distance d / d-1)
    MSLICE = [(128, 192), (0, 192), (0, 192), (0, 192), (0, 128)]

    with tile.TileContext(nc) as tc:
        with (
            tc.tile_pool(name="dram", bufs=1, space="DRAM") as dram,
            tc.tile_pool(name="big", bufs=1) as big,
            tc.tile_pool(name="xin", bufs=4) as xin_pool,
            tc.tile_pool(name="xsc", bufs=4) as xsc_pool,
            tc.tile_pool(name="wqk", bufs=4) as wqk_pool,
            tc.tile_pool(name="wvp", bufs=2) as wv_pool,
            tc.tile_pool(name="pt", bufs=34) as pt_pool,
            tc.tile_pool(name="oacc", bufs=1) as oacc_pool,
            tc.tile_pool(name="oq", bufs=4) as oq_pool,
            tc.tile_pool(name="rec", bufs=4) as rec_pool,
            tc.tile_pool(name="misc", bufs=1) as misc,
            tc.tile_pool(name="pss", bufs=4, space="PSUM") as ps_small,
            tc.tile_pool(name="psp", bufs=2, space="PSUM") as ps_proj,
            tc.tile_pool(name="pso", bufs=2, space="PSUM") as ps_o,
        ):
            # --- weight all-gather: 1/8 int8 row shards -> full [HID, HID];
            # weights are per-hid-row int8 (amax/127), dequantized to bf16
            # at tile load with per-partition ACT scales (hid sits on
            # partitions in all three projection matmul layouts) ---
            wfull = []
            for name, wsh in (("wq", wqs), ("wk", wks), ("wv", wvs)):
                bounce = dram.tile([WSH, HID], I8, tag=f"{name}b")
                full = dram.tile([HID, HID], I8, tag=f"{name}f")
                nc.sync.dma_start(out=bounce[:], in_=wsh[:])
                nc.gpsimd.collective_compute(
                    "AllGather", mybir.AluOpType.bypass,
                    replica_groups=[list(range(CORES))],
                    ins=[bounce.opt()], outs=[full.opt()])
                wfull.append(full)
            wq_full, wk_full, wv_full = wfull
            wsc_sb = []
            for m in range(3):
                wss = misc.tile([128, 8], F32, tag=f"wsc{m}")
                nc.sync.dma_start(
                    out=wss[:],
                    in_=wsc[m, :].rearrange("(ht p) -> p ht", p=128))
                wsc_sb.append(wss)

            # constants generated on device (saves wire bytes): identity for
            # PE transposes and the mask family mgen = [D0|D1|D2] where
            # block Dd holds the masks for chunk-distance d (top 64 rows)
            # and d-1 (bottom 64 rows): 0 -> causal, 1 -> ones, else zeros.
            ones_sb = misc.tile([128, 128], BF16, tag="ones")
            nc.vector.memset(ones_sb[:], 1.0)
            ident_sb = misc.tile([128, 128], BF16, tag="ident")
            nc.gpsimd.affine_select(
                ident_sb[:], ones_sb[:], pattern=[[1, 128]],
                compare_op=mybir.AluOpType.is_equal, fill=0.0,
                base=0, channel_multiplier=-1)
            mgen_sb = misc.tile([128, 192], BF16, tag="mgen")
            # D0: f - p >= 0 -> top causal, bottom (p>=64) all fill(0)
            nc.gpsimd.affine_select(
                mgen_sb[:, 0:64], ones_sb[:, 0:64], pattern=[[1, 64]],
                compare_op=mybir.AluOpType.is_ge, fill=0.0,
                base=0, channel_multiplier=-1)
            # D1: f - p + 64 >= 0 -> top all ones, bottom causal
            nc.gpsimd.affine_select(
                mgen_sb[:, 64:128], ones_sb[:, 0:64], pattern=[[1, 64]],
                compare_op=mybir.AluOpType.is_ge, fill=0.0,
                base=64, channel_multiplier=-1)
            # D2: p - 64 >= 0 -> top zeros, bottom all ones
            nc.gpsimd.affine_select(
                mgen_sb[:, 128:192], ones_sb[:, 0:64], pattern=[[0, 64]],
                compare_op=mybir.AluOpType.is_ge, fill=0.0,
                base=-64, channel_multiplier=1)
            # --- halo ring-exchange: each core sends its LAST 64 rows of
            # both batches (dequantized bf16, rows on partitions: batch b at
            # partitions [64b, 64b+64)) to the next core. SPMD-safe slot
            # selection: slot j of the RS input carries the halo iff
            # j == (pid+1) mod 8 (masked multiply by an is_equal of the
            # PE-broadcast partition id); ReduceScatter(add) then hands core
            # i slot i = core (i-1)'s halo. The 64 rows of the front chunk
            # beyond the halo are masked out by mgen/mfirst anyway and are
            # fed as zeros.
            rs_in = dram.tile([CORES, 128, HID], BF16, tag="rsin")
            rs_out = dram.tile([128, HID], BF16, tag="rsout")
            hq = misc.tile([128, HID], I8, tag="hq")
            hsc = misc.tile([128, 1], F32, tag="hsc")
            hbf = misc.tile([128, HID], BF16, tag="hbf")
            for b in range(B):
                nc.sync.dma_start(out=hq[64 * b:64 * b + 64, :],
                                  in_=xq[b, SLICE - 64:SLICE, :])
                nc.sync.dma_start(out=hsc[64 * b:64 * b + 64, :],
                                  in_=xs[b, SLICE - 64:SLICE, :])
            nc.scalar.activation(hbf[:], hq[:], COPY, scale=hsc[:])
            pid_u = misc.tile([1, 1], mybir.dt.uint32, tag="pidu")
            nc.sync.dma_start(out=pid_u[:],
                              in_=nc.partition_id_tensor[0:1, 0:1])
            pid_b = misc.tile([1, 1], BF16, tag="pidb")
            nc.vector.tensor_copy(pid_b[:], pid_u[:])
            pidp = ps_small.tile([128, 192], F32, tag="pp", name="pidp")
            nc.tensor.matmul(pidp[:, 0:1], ones_sb[0:1, :], pid_b[:],
                             start=True, stop=True)
            pidv = misc.tile([128, 1], F32, tag="pidv")
            nc.vector.tensor_scalar_add(pidv[:], pidp[:, 0:1], 1.0)
            # mfirst = D2 slice of mgen, zeroed on core 0 (no wrap-attend
            # for the first chunk of the sequence)
            selm = misc.tile([128, 1], BF16, tag="selm")
            nc.vector.tensor_scalar(selm[:], pidv[:], 1.5, None,
                                    op0=mybir.AluOpType.is_ge)
            mfirst_sb = misc.tile([128, 64], BF16, tag="mfirst")
            nc.vector.tensor_tensor(mfirst_sb[:], mgen_sb[:, 128:192],
                                    selm[:].to_broadcast((128, 64)),
                                    mybir.AluOpType.mult)
            for j in range(CORES):
                sel = rec_pool.tile([128, 1], BF16, tag="sel", name="sel")
                nc.vector.tensor_scalar(
                    sel[:], pidv[:], float(j) if j else 8.0, None,
                    op0=mybir.AluOpType.is_equal)
                slot = oq_pool.tile([128, HID], BF16, tag="slot",
                                    name="slot")
                nc.vector.tensor_tensor(
                    slot[:], hbf[:], sel[:].to_broadcast((128, HID)),
                    mybir.AluOpType.mult)
                nc.sync.dma_start(out=rs_in[j], in_=slot[:])
            nc.gpsimd.collective_compute(
                "ReduceScatter", mybir.AluOpType.add,
                replica_groups=[list(range(CORES))],
                ins=[rs_in.opt()], outs=[rs_out.opt()])

            for b in range(B):
                XT = big.tile([128, 8, SLAB], BF16, tag="xt")
                QT = big.tile([128, 8, SLICE], BF16, tag="qt")
                KT = big.tile([128, 8, SLAB], BF16, tag="kt")
                V1 = big.tile([128, NRT, NH, DH + 1], BF16, tag="v1")
                nc.vector.memset(V1[:, :, :, DH:DH + 1], 1.0)

                # --- Phase A: load int8 + dequant + transpose X ---
                # slab tile 0 = [64 zero rows | 64 ring-exchanged halo rows];
                # tiles 1..8 come from this core's own 1024 rows.
                for rt in range(NRT):
                    xin = xin_pool.tile([128, HID], BF16, tag="xin",
                                        name="xin")
                    if rt == 0:
                        nc.vector.memset(xin[0:64, :], 0.0)
                        nc.sync.dma_start(out=xin[64:128, :],
                                          in_=rs_out[64 * b:64 * b + 64, :])
                    else:
                        r0 = 128 * rt - 128
                        xin8 = xin_pool.tile([128, HID], I8, tag="xin8",
                                             name="xin8")
                        nc.sync.dma_start(out=xin8[:],
                                          in_=xq[b, r0:r0 + 128, :])
                        xsc = xsc_pool.tile([128, 1], F32, tag="xsc")
                        nc.sync.dma_start(out=xsc[:],
                                          in_=xs[b, r0:r0 + 128, :])
                        nc.scalar.activation(xin[:], xin8[:], COPY,
                                             scale=xsc[:])
                    for hp in range(4):
                        # transpose passes through lhsT dtype -> bf16 psum;
                        # full-bank alloc keeps the pool slot size uniform
                        tpf = ps_proj.tile([128, 1024], BF16, tag="proj",
                                           name="tp")
                        tp = tpf[:, 0:256]
                        tm1 = nc.tensor.matmul(
                            tp[:, 0:128], xin[:, 256 * hp:256 * hp + 128],
                            ident_sb[:], is_transpose=True,
                            start=True, stop=False)
                        tm2 = nc.tensor.matmul(
                            tp[:, 128:256],
                            xin[:, 256 * hp + 128:256 * hp + 256],
                            ident_sb[:], is_transpose=True,
                            start=False, stop=True)
                        add_dep_helper(tm2.ins, tm1.ins, sync=False,
                                       reason="psum group order")
                        nc.vector.tensor_copy(
                            XT[:, 2 * hp:2 * hp + 2,
                               128 * rt:128 * rt + 128], tp[:])

                # --- Phase B: projections ---
                # QT: lhsT = wq tile [hid, outd], rhs = XT -> [outd, row] bf16
                for ot in range(8):
                    wt8 = wqk_pool.tile([128, 8, 128], I8, tag="wqk8",
                                        name="wt8")
                    nc.sync.dma_start(
                        out=wt8[:],
                        in_=wq_full[:, 128 * ot:128 * ot + 128].rearrange(
                            "(ht p) o -> p ht o", p=128))
                    wt = wqk_pool.tile([128, 8, 128], BF16, tag="wqk")
                    for ht in range(8):
                        nc.scalar.activation(wt[:, ht, :], wt8[:, ht, :],
                                             COPY,
                                             scale=wsc_sb[0][:, ht:ht + 1])
                    for half in range(2):
                        qp = ps_proj.tile([128, 512], F32, tag="proj")
                        for ht in range(8):
                            nc.tensor.matmul(
                                qp[:], wt[:, ht, :],
                                XT[:, ht, HALO + 512 * half:
                                   HALO + 512 * half + 512],
                                start=(ht == 0), stop=(ht == 7))
                        nc.vector.tensor_copy(
                            QT[:, ot, 512 * half:512 * half + 512], qp[:])

                # KT: same, over all SLAB cols (K pre-scaled on host)
                for ot in range(8):
                    wt8 = wqk_pool.tile([128, 8, 128], I8, tag="wqk8",
                                        name="wt8")
                    nc.sync.dma_start(
                        out=wt8[:],
                        in_=wk_full[:, 128 * ot:128 * ot + 128].rearrange(
                            "(ht p) o -> p ht o", p=128))
                    wt = wqk_pool.tile([128, 8, 128], BF16, tag="wqk")
                    for ht in range(8):
                        nc.scalar.activation(wt[:, ht, :], wt8[:, ht, :],
                                             COPY,
                                             scale=wsc_sb[1][:, ht:ht + 1])
                    for ks in range(SLAB // KS):
                        kpf = ps_proj.tile([128, 512], F32, tag="proj",
                                           name="kpf")
                        kp = kpf[:, 0:KS]
                        for ht in range(8):
                            nc.tensor.matmul(
                                kp[:], wt[:, ht, :],
                                XT[:, ht, KS * ks:KS * ks + KS],
                                start=(ht == 0), stop=(ht == 7))
                        nc.vector.tensor_copy(
                            KT[:, ot, KS * ks:KS * ks + KS], kp[:])

                # V: lhsT = XT row tile, rhs = wv [hid, outd] -> [row, outd]
                for oh in range(2):
                    wvt8 = wv_pool.tile([128, 8, 512], I8, tag="wv8",
                                        name="wvt8")
                    nc.sync.dma_start(
                        out=wvt8[:],
                        in_=wv_full[:, 512 * oh:512 * oh + 512].rearrange(
                            "(ht p) o -> p ht o", p=128))
                    wvt = wv_pool.tile([128, 8, 512], BF16, tag="wv")
                    for ht in range(8):
                        nc.scalar.activation(wvt[:, ht, :], wvt8[:, ht, :],
                                             COPY,
                                             scale=wsc_sb[2][:, ht:ht + 1])
                    for rt in range(NRT):
                        vp = ps_proj.tile([128, 512], F32, tag="proj")
                        for ht in range(8):
                            nc.tensor.matmul(
                                vp[:], XT[:, ht, 128 * rt:128 * rt + 128],
                                wvt[:, ht, :], start=(ht == 0),
                                stop=(ht == 7))
                        nc.vector.tensor_copy(
                            V1[:, rt, 8 * oh:8 * oh + 8, 0:DH], vp[:])

                # --- Phase C: attention ---
                for sp in range(NSP):
                    oacc = oacc_pool.tile([128, 4, HID], F32, tag="oacc")

                    def emit_mm1s(sp, t):
                        pts = {}
                        for l in (1, 0, 2, 3, 4):
                            rt = 4 * sp + l
                            lo, hi = SPANS[l]
                            pps = []
                            for e in range(2):
                                pp = ps_small.tile([128, 192], F32,
                                                   tag="pp", name="pp")
                                nc.tensor.matmul(
                                    pp[:, 0:hi - lo],
                                    KT[64 * e:64 * e + 64, t,
                                       128 * rt:128 * rt + 128],
                                    QT[64 * e:64 * e + 64, t,
                                       512 * sp + lo:512 * sp + hi],
                                    start=True, stop=True,
                                    tile_position=(64 * e, 0))
                                pps.append(pp)
                            for e in range(2):
                                pt = pt_pool.tile([128, 192], BF16, tag="pt",
                                                  name="pt")
                                nc.scalar.activation(pt[:, 0:hi - lo],
                                                     pps[e][:, 0:hi - lo],
                                                     EXP)
                                if l == 0 and sp == 0:
                                    msk = mfirst_sb[:]
                                else:
                                    ml, mh = MSLICE[l]
                                    msk = mgen_sb[:, ml:mh]
                                nc.vector.tensor_tensor(
                                    pt[:, 0:hi - lo], pt[:, 0:hi - lo], msk,
                                    mybir.AluOpType.mult)
                                pts[(e, l)] = pt
                        return pts

                    def emit_pv(sp, t, pts):
                        for e in range(2):
                            h = 2 * t + e
                            # O[qi, d] directly: lhsT = PT slice (qi block on
                            # psum partitions), rhs = [V|1]; all 4 qi blocks
                            # share one psum bank; per block the full-window
                            # tile writes first, the half-window accumulates.
                            ops = ps_o.tile([128, 4, DH + 1], F32, tag="o",
                                            name="ops")
                            prev = None
                            for i, (c4, l, plo, phi) in enumerate(PV_O2):
                                rt = 4 * sp + l
                                mm = nc.tensor.matmul(
                                    ops[0:phi - plo, c4, :],
                                    pts[(e, l)][:, plo:phi],
                                    V1[:, rt, h, :],
                                    start=(i == 0),
                                    stop=(i >= len(PV_O2) - 2),
                                    skip_group_check=True)
                                if prev is not None:
                                    # keep the per-block psum groups in
                                    # program order (flag-clear before the
                                    # next group's start)
                                    add_dep_helper(mm.ins, prev.ins,
                                                   sync=False,
                                                   reason="psum group order")
                                prev = mm
                            rec = rec_pool.tile([128, 4], F32, tag="rec")
                            nc.vector.reciprocal(rec[:], ops[:, :, DH:DH + 1])
                            nc.vector.tensor_tensor(
                                oacc[:, :, DH * h:DH * h + DH],
                                ops[:, :, 0:DH],
                                rec[:, :, None].to_broadcast((128, 4, DH)),
                                mybir.AluOpType.mult)

                    pending = []
                    for t in range(NH // 2):
                        pts = emit_mm1s(sp, t)
                        pending.append((t, pts))
                        if len(pending) > 2:
                            pt_, pts_ = pending.pop(0)
                            emit_pv(sp, pt_, pts_)
                    for pt_, pts_ in pending:
                        emit_pv(sp, pt_, pts_)

                    # int8-quantize the subpanel per out row: amax over hid,
                    # scale = amax/126 (margin vs reciprocal rounding), then
                    # round-to-nearest via the +1.5*2^23 float trick.
                    RB = 12582912.0  # 1.5 * 2**23
                    qam = rec_pool.tile([128, 4], F32, tag="qam", name="qam")
                    nc.vector.tensor_reduce(qam[:], oacc[:],
                                            mybir.AxisListType.X,
                                            mybir.AluOpType.max,
                                            apply_absolute_value=True)
                    oscs = rec_pool.tile([128, 4], F32, tag="oscs",
                                         name="oscs")
                    nc.vector.tensor_scalar_mul(oscs[:], qam[:], 1.0 / 126.0)
                    qrec = rec_pool.tile([128, 4], F32, tag="qrec",
                                         name="qrec")
                    nc.vector.reciprocal(qrec[:], oscs[:])
                    for c4 in range(4):
                        r0 = 512 * sp + 128 * c4
                        t1 = oq_pool.tile([128, HID], F32, tag="t1",
                                          name="t1")
                        nc.vector.tensor_scalar(
                            t1[:], oacc[:, c4, :], qrec[:, c4:c4 + 1], RB,
                            op0=mybir.AluOpType.mult,
                            op1=mybir.AluOpType.add)
                        q8 = oq_pool.tile([128, HID], I8, tag="q8",
                                          name="q8")
                        nc.vector.tensor_scalar(
                            q8[:], t1[:], RB, None,
                            op0=mybir.AluOpType.subtract)
                        nc.sync.dma_start(out=out[b, r0:r0 + 128, :],
                                          in_=q8[:])
                        nc.sync.dma_start(out=osc[b, r0:r0 + 128, :],
                                          in_=oscs[:, c4:c4 + 1])
    nc.compile()
    return nc


def _inputs_for_core(i, xq8, xsc, wq, wk, wv, wscales):
    return {
        "xq": xq8[:, SLICE * i:SLICE * (i + 1)],
        "xs": xsc[:, SLICE * i:SLICE * (i + 1), None],
        "wqs": wq[WSH * i:WSH * (i + 1)],
        "wks": wk[WSH * i:WSH * (i + 1)],
        "wvs": wv[WSH * i:WSH * (i + 1)],
        "wsc": wscales,
    }


def kernel(hidden_states, Wq, Wk, Wv, _trace=False):
    from concourse.bass_utils import run_bass_kernel_spmd

    hidden_states = np.asarray(hidden_states, dtype=np.float32)

    def _wquant(W, pre=1.0):
        W = np.asarray(W, dtype=np.float32) * np.float32(pre)
        am = np.maximum(np.abs(W).max(axis=1), np.float32(1e-20))  # [HID]
        sc = am * np.float32(1.0 / 127.0)
        q = np.rint(W * (np.float32(1.0) / sc)[:, None]).astype(np.int8)
        return q, sc

    Wq, sq = _wquant(Wq)
    Wk, sk = _wquant(Wk, pre=1.0 / np.sqrt(DH))
    Wv, sv = _wquant(Wv)
    wsc = np.stack([sq, sk, sv]).astype(np.float32)  # [3, HID]

    # per-row symmetric int8 quantization of hidden_states; numpy releases
    # the GIL on large array ops so chunked threads give real speedup
    if "pool" not in _CACHE:
        _CACHE["pool"] = ThreadPoolExecutor(max_workers=8)
        _CACHE["qtmp"] = np.empty((B, S, HID), dtype=np.float32)
        _CACHE["q8"] = np.empty((B, S, HID), dtype=np.int8)
    pool = _CACHE["pool"]
    tmp, xq8 = _CACHE["qtmp"], _CACHE["q8"]
    xsc = np.empty((B, S), dtype=np.float32)

    def _quant_chunk(b, s0, s1):
        h = hidden_states[b, s0:s1]
        t = tmp[b, s0:s1]
        am = np.maximum(np.maximum(h.max(axis=-1), -h.min(axis=-1)),
                        np.float32(1e-20))
        xsc[b, s0:s1] = am * np.float32(1.0 / 127.0)
        np.multiply(h, (np.float32(127.0) / am)[:, None], out=t)
        np.rint(t, out=t)
        xq8[b, s0:s1] = t

    CH = S // 4
    list(pool.map(lambda a: _quant_chunk(*a),
                  [(b, c * CH, (c + 1) * CH) for b in range(B)
                   for c in range(4)]))

    if "nc" not in _CACHE:
        _CACHE["nc"] = _build()
    nc = _CACHE["nc"]

    in_maps = [_inputs_for_core(i, xq8, xsc, Wq, Wk, Wv, wsc)
               for i in range(CORES)]
    res = run_bass_kernel_spmd(nc, in_maps, list(range(CORES)), trace=_trace)
    _CACHE["last"] = res
    full = np.empty((B, S, HID), dtype=np.float32)

    def _decode(i):
        r = res.results[i]
        np.multiply(r["out"], r["osc"],
                    out=full[:, SLICE * i:SLICE * (i + 1), :])

    list(_CACHE["pool"].map(_decode, range(CORES)))
    return full


# revision 55
# speedup vs baseline: 1.0233x; 1.0051x over previous
"""Trainium2 Bass kernel for chunked local self-attention (8-core SPMD).

Model (hardcoded from the problem spec):
  B=2, S=8192, HID=1024, NH=16, DH=64, CHUNK=64, N_BEFORE=1, N_AFTER=0,
  decoder-causal, softmax over a 128-wide rolled window per 64-chunk.

Sharding: sequence-parallel over 8 cores. Core i handles seq rows
[1024*i, 1024*(i+1)) of both batches; the 64-row (1-chunk) front halo the
local attention window needs is ring-exchanged on device (wrapped,
matching jnp.roll semantics; the wrapped window is masked out exactly as
in the reference).

Wire-format optimizations (the end-to-end time is dominated by the axon
host<->device tunnel at ~30 MB/s, not device compute):
  - hidden_states are sent as per-row int8 (amax/127 per-row scale),
    dequantized to bf16 on device by the scalar engine; no halo
    duplication (on-device ReduceScatter ring exchange).
  - weights are sent as per-hid-row int8 1/8-row-shards (+ f32 row
    scales) and AllGather'd on device, dequantized to bf16 at tile load
    with per-partition ACT scales (96 MB of replicated f32 -> 3.2 MB).
  - output returns as per-row-int8 + f32 row scales (minimizes both the
    donated zero-init upload and the result download), decoded on host.
  - masks/identity are generated on device (affine_select).

Per-core pipeline (per batch):
  1. DMA int8 X slab rows, ACT-dequant to bf16, PE-transpose to XT
     [hid, row] (bf16).
  2. QKV projections on PE in bf16:
       QT[outd, row], KT[outd, row] (K pre-scaled on host),
       V[row, outd] (+ones col) via lhsT/rhs role swaps of XT.
  3. Attention per (512-row subpanel, head-pair): banded matmuls per
     128-row V tile rt:
       PT_raw[kv, qi] = KT-tile x QT-span   (one MM per tile, kv on psum
                                             partitions; both heads of a
                                             pair run concurrently on
                                             disjoint PE row groups)
       PT = exp(PT_raw) * mask   (ACT exp psum->bf16, DVE mask multiply)
       OT[qi, d] += PT^T x [V|1] (single PSUM accumulator; row 64 gathers
                                  the softmax denominators)
       scale rows by 1/sums into an f32 assembly buffer; per-row amax
       int8 quantization (round-to-nearest via the 1.5*2^23 trick) and
       4+4 batched DMAs (int8 data + f32 row scales) per subpanel.
"""

import sys

sys.path.insert(0, "/opt/trn_rl_repo")

import numpy as np
from concurrent.futures import ThreadPoolExecutor

B, S, HID = 2, 8192, 1024
NH, DH = 16, 64
CHUNK = 64
CORES = 8
SLICE = S // CORES          # 1024 q rows per core per batch
HALO = 128                  # 2-chunk front pad: 1 zero chunk + 1 ring-
                            # exchanged halo chunk (keeps tiles 128-aligned)
SLAB = SLICE + HALO         # 1152 rows of XT/KT per core per batch
NRT = SLAB // 128           # 9 row tiles of V / X
NSP = SLICE // 512          # 2 attention subpanels per batch
KS = 384                    # KT projection free-dim span
WSH = HID // CORES          # 128 weight rows per core shard

_CACHE = {}


def _build():
    import concourse.bass as bass
    import concourse.tile as tile
    from concourse.tile import add_dep_helper
    from concourse import mybir, bacc

    F32 = mybir.dt.float32
    BF16 = mybir.dt.bfloat16
    I8 = mybir.dt.int8
    EXP = mybir.ActivationFunctionType.Exp
    COPY = mybir.ActivationFunctionType.Copy

    nc = bacc.Bacc("TRN2", target_bir_lowering=False, debug=False,
                   num_devices=CORES)

    # all inputs ride in TWO packed blobs (one per dtype): the axon tunnel
    # charges ~60ms of round-trip latency PER ARRAY, so 7 separate inputs
    # cost ~0.3s of pure latency. Layout:
    #   bi8:  [xq (B,SLICE,HID)] [wq8 (WSH,HID)] [wk8] [wv8]
    #   bf32: [xs (B,SLICE)] [wsc (3,HID)]
    XQ_SZ = B * SLICE * HID
    WB_SZ = WSH * HID
    NI8 = XQ_SZ + 3 * WB_SZ
    NF32 = B * SLICE + 3 * HID
    bi8 = nc.dram_tensor("bi8", [NI8], I8, kind="ExternalInput")
    bf32 = nc.dram_tensor("bf32", [NF32], F32, kind="ExternalInput")

    def xq_rows(b, r0, n):  # [n, HID] int8 rows of batch b
        o = (b * SLICE + r0) * HID
        return bi8[o:o + n * HID].rearrange("(p o) -> p o", p=n)

    def xs_rows(b, r0, n):  # [n, 1] f32 row scales
        o = b * SLICE + r0
        return bf32[o:o + n].rearrange("(p o) -> p o", p=n)
    out = nc.dram_tensor("out", [B, SLICE, HID], I8, kind="ExternalOutput")
    osc = nc.dram_tensor("osc", [B, SLICE, 1], F32, kind="ExternalOutput")

    # qi col spans (local to a 512-col subpanel) of the band MM for V-tile
    # l = rt - 4*sp, and the PV accumulation order/splits: (l, lo, hi) with
    # lo/hi in subpanel cols; pt-tile cols are [lo - SPANS[l][0], ...).
    SPANS = [(0, 64), (0, 192), (128, 320), (256, 448), (384, 512)]
    # PV accumulation: (qi block c4, V tile l, pt col lo, pt col hi); per
    # block the full-window tile (M=128) writes first, the half-window
    # (M=64) accumulates onto partitions [0:64). All 8 MMs form one ordered
    # psum group; stop is set on the last M=128 and the last MM so the
    # per-partition group flags clear for the whole bank.
    PV_O2 = [(0, 1, 0, 128), (0, 0, 0, 64),
             (1, 2, 0, 128), (1, 1, 128, 192),
             (2, 3, 0, 128), (2, 2, 128, 192),
             (3, 4, 0, 128), (3, 3, 128, 192)]
    # mask slice of mgen [128, 192] = [D0|D1|D2] per l (mgen is generated
    # on device below: block Dd = masks for chunk-distance d / d-1)
    MSLICE = [(128, 192), (0, 192), (0, 192), (0, 192), (0, 128)]

    with tile.TileContext(nc) as tc:
        with (
            tc.tile_pool(name="dram", bufs=1, space="DRAM") as dram,
            tc.tile_pool(name="big", bufs=1) as big,
            tc.tile_pool(name="xin", bufs=4) as xin_pool,
            tc.tile_pool(name="xsc", bufs=4) as xsc_pool,
            tc.tile_pool(name="wqk", bufs=4) as wqk_pool,
            tc.tile_pool(name="wvp", bufs=2) as wv_pool,
            tc.tile_pool(name="pt", bufs=34) as pt_pool,
            tc.tile_pool(name="oacc", bufs=1) as oacc_pool,
            tc.tile_pool(name="oq", bufs=4) as oq_pool,
            tc.tile_pool(name="rec", bufs=4) as rec_pool,
            tc.tile_pool(name="misc", bufs=1) as misc,
            tc.tile_pool(name="pss", bufs=4, space="PSUM") as ps_small,
            tc.tile_pool(name="psp", bufs=2, space="PSUM") as ps_proj,
            tc.tile_pool(name="pso", bufs=2, space="PSUM") as ps_o,
        ):
            # --- weight all-gather: 1/8 int8 row shards -> full [HID, HID];
            # weights are per-hid-row int8 (amax/127), dequantized to bf16
            # at tile load with per-partition ACT scales (hid sits on
            # partitions in all three projection matmul layouts) ---
            wfull = []
            for m, name in enumerate(("wq", "wk", "wv")):
                bounce = dram.tile([WSH, HID], I8, tag=f"{name}b")
                full = dram.tile([HID, HID], I8, tag=f"{name}f")
                o = XQ_SZ + m * WB_SZ
                nc.sync.dma_start(
                    out=bounce[:],
                    in_=bi8[o:o + WB_SZ].rearrange("(p o) -> p o", p=WSH))
                nc.gpsimd.collective_compute(
                    "AllGather", mybir.AluOpType.bypass,
                    replica_groups=[list(range(CORES))],
                    ins=[bounce.opt()], outs=[full.opt()])
                wfull.append(full)
            wq_full, wk_full, wv_full = wfull
            wsc_sb = []
            for m in range(3):
                wss = misc.tile([128, 8], F32, tag=f"wsc{m}")
                o = B * SLICE + m * HID
                nc.sync.dma_start(
                    out=wss[:],
                    in_=bf32[o:o + HID].rearrange("(ht p) -> p ht", p=128))
                wsc_sb.append(wss)

            # constants generated on device (saves wire bytes): identity for
            # PE transposes and the mask family mgen = [D0|D1|D2] where
            # block Dd holds the masks for chunk-distance d (top 64 rows)
            # and d-1 (bottom 64 rows): 0 -> causal, 1 -> ones, else zeros.
            ones_sb = misc.tile([128, 128], BF16, tag="ones")
            nc.vector.memset(ones_sb[:], 1.0)
            ident_sb = misc.tile([128, 128], BF16, tag="ident")
            nc.gpsimd.affine_select(
                ident_sb[:], ones_sb[:], pattern=[[1, 128]],
                compare_op=mybir.AluOpType.is_equal, fill=0.0,
                base=0, channel_multiplier=-1)
            mgen_sb = misc.tile([128, 192], BF16, tag="mgen")
            # D0: f - p >= 0 -> top causal, bottom (p>=64) all fill(0)
            nc.gpsimd.affine_select(
                mgen_sb[:, 0:64], ones_sb[:, 0:64], pattern=[[1, 64]],
                compare_op=mybir.AluOpType.is_ge, fill=0.0,
                base=0, channel_multiplier=-1)
            # D1: f - p + 64 >= 0 -> top all ones, bottom causal
            nc.gpsimd.affine_select(
                mgen_sb[:, 64:128], ones_sb[:, 0:64], pattern=[[1, 64]],
                compare_op=mybir.AluOpType.is_ge, fill=0.0,
                base=64, channel_multiplier=-1)
            # D2: p - 64 >= 0 -> top zeros, bottom all ones
            nc.gpsimd.affine_select(
                mgen_sb[:, 128:192], ones_sb[:, 0:64], pattern=[[0, 64]],
                compare_op=mybir.AluOpType.is_ge, fill=0.0,
                base=-64, channel_multiplier=1)
            # --- halo ring-exchange: each core sends its LAST 64 rows of
            # both batches (dequantized bf16, rows on partitions: batch b at
            # partitions [64b, 64b+64)) to the next core. SPMD-safe slot
            # selection: slot j of the RS input carries the halo iff
            # j == (pid+1) mod 8 (masked multiply by an is_equal of the
            # PE-broadcast partition id); ReduceScatter(add) then hands core
            # i slot i = core (i-1)'s halo. The 64 rows of the front chunk
            # beyond the halo are masked out by mgen/mfirst anyway and are
            # fed as zeros.
            rs_in = dram.tile([CORES, 128, HID], BF16, tag="rsin")
            rs_out = dram.tile([128, HID], BF16, tag="rsout")
            hq = misc.tile([128, HID], I8, tag="hq")
            hsc = misc.tile([128, 1], F32, tag="hsc")
            hbf = misc.tile([128, HID], BF16, tag="hbf")
            for b in range(B):
                nc.sync.dma_start(out=hq[64 * b:64 * b + 64, :],
                                  in_=xq_rows(b, SLICE - 64, 64))
                nc.sync.dma_start(out=hsc[64 * b:64 * b + 64, :],
                                  in_=xs_rows(b, SLICE - 64, 64))
            nc.scalar.activation(hbf[:], hq[:], COPY, scale=hsc[:])
            pid_u = misc.tile([1, 1], mybir.dt.uint32, tag="pidu")
            nc.sync.dma_start(out=pid_u[:],
                              in_=nc.partition_id_tensor[0:1, 0:1])
            pid_b = misc.tile([1, 1], BF16, tag="pidb")
            nc.vector.tensor_copy(pid_b[:], pid_u[:])
            pidp = ps_small.tile([128, 192], F32, tag="pp", name="pidp")
            nc.tensor.matmul(pidp[:, 0:1], ones_sb[0:1, :], pid_b[:],
                             start=True, stop=True)
            pidv = misc.tile([128, 1], F32, tag="pidv")
            nc.vector.tensor_scalar_add(pidv[:], pidp[:, 0:1], 1.0)
            # mfirst = D2 slice of mgen, zeroed on core 0 (no wrap-attend
            # for the first chunk of the sequence)
            selm = misc.tile([128, 1], BF16, tag="selm")
            nc.vector.tensor_scalar(selm[:], pidv[:], 1.5, None,
                                    op0=mybir.AluOpType.is_ge)
            mfirst_sb = misc.tile([128, 64], BF16, tag="mfirst")
            nc.vector.tensor_tensor(mfirst_sb[:], mgen_sb[:, 128:192],
                                    selm[:].to_broadcast((128, 64)),
                                    mybir.AluOpType.mult)
            for j in range(CORES):
                sel = rec_pool.tile([128, 1], BF16, tag="sel", name="sel")
                nc.vector.tensor_scalar(
                    sel[:], pidv[:], float(j) if j else 8.0, None,
                    op0=mybir.AluOpType.is_equal)
                slot = oq_pool.tile([128, HID], BF16, tag="slot",
                                    name="slot")
                nc.vector.tensor_tensor(
                    slot[:], hbf[:], sel[:].to_broadcast((128, HID)),
                    mybir.AluOpType.mult)
                nc.sync.dma_start(out=rs_in[j], in_=slot[:])
            nc.gpsimd.collective_compute(
                "ReduceScatter", mybir.AluOpType.add,
                replica_groups=[list(range(CORES))],
                ins=[rs_in.opt()], outs=[rs_out.opt()])

            for b in range(B):
                XT = big.tile([128, 8, SLAB], BF16, tag="xt")
                QT = big.tile([128, 8, SLICE], BF16, tag="qt")
                KT = big.tile([128, 8, SLAB], BF16, tag="kt")
                V1 = big.tile([128, NRT, NH, DH + 1], BF16, tag="v1")
                nc.vector.memset(V1[:, :, :, DH:DH + 1], 1.0)

                # --- Phase A: load int8 + dequant + transpose X ---
                # slab tile 0 = [64 zero rows | 64 ring-exchanged halo rows];
                # tiles 1..8 come from this core's own 1024 rows.
                for rt in range(NRT):
                    xin = xin_pool.tile([128, HID], BF16, tag="xin",
                                        name="xin")
                    if rt == 0:
                        nc.vector.memset(xin[0:64, :], 0.0)
                        nc.sync.dma_start(out=xin[64:128, :],
                                          in_=rs_out[64 * b:64 * b + 64, :])
                    else:
                        r0 = 128 * rt - 128
                        xin8 = xin_pool.tile([128, HID], I8, tag="xin8",
                                             name="xin8")
                        nc.sync.dma_start(out=xin8[:],
                                          in_=xq_rows(b, r0, 128))
                        xsc = xsc_pool.tile([128, 1], F32, tag="xsc")
                        nc.sync.dma_start(out=xsc[:],
                                          in_=xs_rows(b, r0, 128))
                        nc.scalar.activation(xin[:], xin8[:], COPY,
                                             scale=xsc[:])
                    for hp in range(4):
                        # transpose passes through lhsT dtype -> bf16 psum;
                        # full-bank alloc keeps the pool slot size uniform
                        tpf = ps_proj.tile([128, 1024], BF16, tag="proj",
                                           name="tp")
                        tp = tpf[:, 0:256]
                        tm1 = nc.tensor.matmul(
                            tp[:, 0:128], xin[:, 256 * hp:256 * hp + 128],
                            ident_sb[:], is_transpose=True,
                            start=True, stop=False)
                        tm2 = nc.tensor.matmul(
                            tp[:, 128:256],
                            xin[:, 256 * hp + 128:256 * hp + 256],
                            ident_sb[:], is_transpose=True,
                            start=False, stop=True)
                        add_dep_helper(tm2.ins, tm1.ins, sync=False,
                                       reason="psum group order")
                        nc.vector.tensor_copy(
                            XT[:, 2 * hp:2 * hp + 2,
                               128 * rt:128 * rt + 128], tp[:])

                # --- Phase B: projections ---
                # QT: lhsT = wq tile [hid, outd], rhs = XT -> [outd, row] bf16
                for ot in range(8):
                    wt8 = wqk_pool.tile([128, 8, 128], I8, tag="wqk8",
                                        name="wt8")
                    nc.sync.dma_start(
                        out=wt8[:],
                        in_=wq_full[:, 128 * ot:128 * ot + 128].rearrange(
                            "(ht p) o -> p ht o", p=128))
                    wt = wqk_pool.tile([128, 8, 128], BF16, tag="wqk")
                    for ht in range(8):
                        nc.scalar.activation(wt[:, ht, :], wt8[:, ht, :],
                                             COPY,
                                             scale=wsc_sb[0][:, ht:ht + 1])
                    for half in range(2):
                        qp = ps_proj.tile([128, 512], F32, tag="proj")
                        for ht in range(8):
                            nc.tensor.matmul(
                                qp[:], wt[:, ht, :],
                                XT[:, ht, HALO + 512 * half:
                                   HALO + 512 * half + 512],
                                start=(ht == 0), stop=(ht == 7))
                        nc.vector.tensor_copy(
                            QT[:, ot, 512 * half:512 * half + 512], qp[:])

                # KT: same, over all SLAB cols (K pre-scaled on host)
                for ot in range(8):
                    wt8 = wqk_pool.tile([128, 8, 128], I8, tag="wqk8",
                                        name="wt8")
                    nc.sync.dma_start(
                        out=wt8[:],
                        in_=wk_full[:, 128 * ot:128 * ot + 128].rearrange(
                            "(ht p) o -> p ht o", p=128))
                    wt = wqk_pool.tile([128, 8, 128], BF16, tag="wqk")
                    for ht in range(8):
                        nc.scalar.activation(wt[:, ht, :], wt8[:, ht, :],
                                             COPY,
                                             scale=wsc_sb[1][:, ht:ht + 1])
                    for ks in range(SLAB // KS):
                        kpf = ps_proj.tile([128, 512], F32, tag="proj",
                                           name="kpf")
                        kp = kpf[:, 0:KS]
                        for ht in range(8):
                            nc.tensor.matmul(
                                kp[:], wt[:, ht, :],
                                XT[:, ht, KS * ks:KS * ks + KS],
                                start=(ht == 0), stop=(ht == 7))
                        nc.vector.tensor_copy(
                            KT[:, ot, KS * ks:KS * ks + KS], kp[:])

                # V: lhsT = XT row tile, rhs = wv [hid, outd] -> [row, outd]
                for oh in range(2):
                    wvt8 = wv_pool.tile([128, 8, 512], I8, tag="wv8",
                                        name="wvt8")
                    nc.sync.dma_start(
                        out=wvt8[:],
                        in_=wv_full[:, 512 * oh:512 * oh + 512].rearrange(
                            "(ht p) o -> p ht o", p=128))
                    wvt = wv_pool.tile([128, 8, 512], BF16, tag="wv")
                    for ht in range(8):
                        nc.scalar.activation(wvt[:, ht, :], wvt8[:, ht, :],
                                             COPY,
                                             scale=wsc_sb[2][:, ht:ht + 1])
                    for rt in range(NRT):
                        vp = ps_proj.tile([128, 512], F32, tag="proj")
                        for ht in range(8):
                            nc.tensor.matmul(
                                vp[:], XT[:, ht, 128 * rt:128 * rt + 128],
                                wvt[:, ht, :], start=(ht == 0),
                                stop=(ht == 7))
                        nc.vector.tensor_copy(
                            V1[:, rt, 8 * oh:8 * oh + 8, 0:DH], vp[:])

                # --- Phase C: attention ---
                for sp in range(NSP):
                    oacc = oacc_pool.tile([128, 4, HID], F32, tag="oacc")

                    def emit_mm1s(sp, t):
                        pts = {}
                        for l in (1, 0, 2, 3, 4):
                            rt = 4 * sp + l
                            lo, hi = SPANS[l]
                            pps = []
                            for e in range(2):
                                pp = ps_small.tile([128, 192], F32,
                                                   tag="pp", name="pp")
                                nc.tensor.matmul(
                                    pp[:, 0:hi - lo],
                                    KT[64 * e:64 * e + 64, t,
                                       128 * rt:128 * rt + 128],
                                    QT[64 * e:64 * e + 64, t,
                                       512 * sp + lo:512 * sp + hi],
                                    start=True, stop=True,
                                    tile_position=(64 * e, 0))
                                pps.append(pp)
                            for e in range(2):
                                pt = pt_pool.tile([128, 192], BF16, tag="pt",
                                                  name="pt")
                                nc.scalar.activation(pt[:, 0:hi - lo],
                                                     pps[e][:, 0:hi - lo],
                                                     EXP)
                                if l == 0 and sp == 0:
                                    msk = mfirst_sb[:]
                                else:
                                    ml, mh = MSLICE[l]
                                    msk = mgen_sb[:, ml:mh]
                                nc.vector.tensor_tensor(
                                    pt[:, 0:hi - lo], pt[:, 0:hi - lo], msk,
                                    mybir.AluOpType.mult)
                                pts[(e, l)] = pt
                        return pts

                    def emit_pv(sp, t, pts):
                        for e in range(2):
                            h = 2 * t + e
                            # O[qi, d] directly: lhsT = PT slice (qi block on
                            # psum partitions), rhs = [V|1]; all 4 qi blocks
                            # share one psum bank; per block the full-window
                            # tile writes first, the half-window accumulates.
                            ops = ps_o.tile([128, 4, DH + 1], F32, tag="o",
                                            name="ops")
                            prev = None
                            for i, (c4, l, plo, phi) in enumerate(PV_O2):
                                rt = 4 * sp + l
                                mm = nc.tensor.matmul(
                                    ops[0:phi - plo, c4, :],
                                    pts[(e, l)][:, plo:phi],
                                    V1[:, rt, h, :],
                                    start=(i == 0),
                                    stop=(i >= len(PV_O2) - 2),
                                    skip_group_check=True)
                                if prev is not None:
                                    # keep the per-block psum groups in
                                    # program order (flag-clear before the
                                    # next group's start)
                                    add_dep_helper(mm.ins, prev.ins,
                                                   sync=False,
                                                   reason="psum group order")
                                prev = mm
                            rec = rec_pool.tile([128, 4], F32, tag="rec")
                            nc.vector.reciprocal(rec[:], ops[:, :, DH:DH + 1])
                            nc.vector.tensor_tensor(
                                oacc[:, :, DH * h:DH * h + DH],
                                ops[:, :, 0:DH],
                                rec[:, :, None].to_broadcast((128, 4, DH)),
                                mybir.AluOpType.mult)

                    pending = []
                    for t in range(NH // 2):
                        pts = emit_mm1s(sp, t)
                        pending.append((t, pts))
                        if len(pending) > 2:
                            pt_, pts_ = pending.pop(0)
                            emit_pv(sp, pt_, pts_)
                    for pt_, pts_ in pending:
                        emit_pv(sp, pt_, pts_)

                    # int8-quantize the subpanel per out row: amax over hid,
                    # scale = amax/126 (margin vs reciprocal rounding), then
                    # round-to-nearest via the +1.5*2^23 float trick.
                    RB = 12582912.0  # 1.5 * 2**23
                    qam = rec_pool.tile([128, 4], F32, tag="qam", name="qam")
                    nc.vector.tensor_reduce(qam[:], oacc[:],
                                            mybir.AxisListType.X,
                                            mybir.AluOpType.max,
                                            apply_absolute_value=True)
                    oscs = rec_pool.tile([128, 4], F32, tag="oscs",
                                         name="oscs")
                    nc.vector.tensor_scalar_mul(oscs[:], qam[:], 1.0 / 126.0)
                    qrec = rec_pool.tile([128, 4], F32, tag="qrec",
                                         name="qrec")
                    nc.vector.reciprocal(qrec[:], oscs[:])
                    for c4 in range(4):
                        r0 = 512 * sp + 128 * c4
                        t1 = oq_pool.tile([128, HID], F32, tag="t1",
                                          name="t1")
                        nc.vector.tensor_scalar(
                            t1[:], oacc[:, c4, :], qrec[:, c4:c4 + 1], RB,
                            op0=mybir.AluOpType.mult,
                            op1=mybir.AluOpType.add)
                        q8 = oq_pool.tile([128, HID], I8, tag="q8",
                                          name="q8")
                        nc.vector.tensor_scalar(
                            q8[:], t1[:], RB, None,
                            op0=mybir.AluOpType.subtract)
                        nc.sync.dma_start(out=out[b, r0:r0 + 128, :],
                                          in_=q8[:])
                        nc.sync.dma_start(out=osc[b, r0:r0 + 128, :],
                                          in_=oscs[:, c4:c4 + 1])
    nc.compile()
    return nc


XQ_SZ = B * SLICE * HID
WB_SZ = WSH * HID
NI8 = XQ_SZ + 3 * WB_SZ
NF32 = B * SLICE + 3 * HID


def _pack_core(i, bi8, bf32, xq8, xsc, wq, wk, wv, wscales):
    bi8[:XQ_SZ].reshape(B, SLICE, HID)[...] = xq8[:, SLICE * i:SLICE * (i + 1)]
    for m, w in enumerate((wq, wk, wv)):
        o = XQ_SZ + m * WB_SZ
        bi8[o:o + WB_SZ].reshape(WSH, HID)[...] = w[WSH * i:WSH * (i + 1)]
    bf32[:B * SLICE].reshape(B, SLICE)[...] = xsc[:, SLICE * i:SLICE * (i + 1)]
    bf32[B * SLICE:].reshape(3, HID)[...] = wscales


def kernel(hidden_states, Wq, Wk, Wv, _trace=False):
    from concourse.bass_utils import run_bass_kernel_spmd

    hidden_states = np.asarray(hidden_states, dtype=np.float32)

    def _wquant(W, pre=1.0):
        W = np.asarray(W, dtype=np.float32) * np.float32(pre)
        am = np.maximum(np.abs(W).max(axis=1), np.float32(1e-20))  # [HID]
        sc = am * np.float32(1.0 / 127.0)
        q = np.rint(W * (np.float32(1.0) / sc)[:, None]).astype(np.int8)
        return q, sc

    Wq, sq = _wquant(Wq)
    Wk, sk = _wquant(Wk, pre=1.0 / np.sqrt(DH))
    Wv, sv = _wquant(Wv)
    wsc = np.stack([sq, sk, sv]).astype(np.float32)  # [3, HID]

    # per-row symmetric int8 quantization of hidden_states; numpy releases
    # the GIL on large array ops so chunked threads give real speedup
    if "pool" not in _CACHE:
        _CACHE["pool"] = ThreadPoolExecutor(max_workers=8)
        _CACHE["qtmp"] = np.empty((B, S, HID), dtype=np.float32)
        _CACHE["q8"] = np.empty((B, S, HID), dtype=np.int8)
    pool = _CACHE["pool"]
    tmp, xq8 = _CACHE["qtmp"], _CACHE["q8"]
    xsc = np.empty((B, S), dtype=np.float32)

    def _quant_chunk(b, s0, s1):
        h = hidden_states[b, s0:s1]
        t = tmp[b, s0:s1]
        am = np.maximum(np.maximum(h.max(axis=-1), -h.min(axis=-1)),
                        np.float32(1e-20))
        xsc[b, s0:s1] = am * np.float32(1.0 / 127.0)
        np.multiply(h, (np.float32(127.0) / am)[:, None], out=t)
        np.rint(t, out=t)
        xq8[b, s0:s1] = t

    CH = S // 4
    list(pool.map(lambda a: _quant_chunk(*a),
                  [(b, c * CH, (c + 1) * CH) for b in range(B)
                   for c in range(4)]))

    if "nc" not in _CACHE:
        _CACHE["nc"] = _build()
    nc = _CACHE["nc"]

    if "bi8" not in _CACHE:
        _CACHE["bi8"] = np.empty((CORES, NI8), dtype=np.int8)
        _CACHE["bf32"] = np.empty((CORES, NF32), dtype=np.float32)
    bi8_all, bf32_all = _CACHE["bi8"], _CACHE["bf32"]
    list(_CACHE["pool"].map(
        lambda i: _pack_core(i, bi8_all[i], bf32_all[i], xq8, xsc,
                             Wq, Wk, Wv, wsc), range(CORES)))
    in_maps = [{"bi8": bi8_all[i], "bf32": bf32_all[i]}
               for i in range(CORES)]
    res = run_bass_kernel_spmd(nc, in_maps, list(range(CORES)), trace=_trace)
    _CACHE["last"] = res
    full = np.empty((B, S, HID), dtype=np.float32)

    def _decode(i):
        r = res.results[i]
        np.multiply(r["out"], r["osc"],
                    out=full[:, SLICE * i:SLICE * (i + 1), :])

    list(_CACHE["pool"].map(_decode, range(CORES)))
    return full


# revision 58
# speedup vs baseline: 1.1205x; 1.0950x over previous
"""Trainium2 Bass kernel for chunked local self-attention (8-core SPMD).

Model (hardcoded from the problem spec):
  B=2, S=8192, HID=1024, NH=16, DH=64, CHUNK=64, N_BEFORE=1, N_AFTER=0,
  decoder-causal, softmax over a 128-wide rolled window per 64-chunk.

Sharding: sequence-parallel over 8 cores. Core i handles seq rows
[1024*i, 1024*(i+1)) of both batches; the 64-row (1-chunk) front halo the
local attention window needs is ring-exchanged on device (wrapped,
matching jnp.roll semantics; the wrapped window is masked out exactly as
in the reference).

Wire-format optimizations (the end-to-end time is dominated by the axon
host<->device tunnel at ~30 MB/s, not device compute):
  - hidden_states are sent as per-row int8 (amax/127 per-row scale),
    dequantized to bf16 on device by the scalar engine; no halo
    duplication (on-device ReduceScatter ring exchange).
  - weights are sent as per-hid-row int8 1/8-row-shards (+ f32 row
    scales) and AllGather'd on device, dequantized to bf16 at tile load
    with per-partition ACT scales (96 MB of replicated f32 -> 3.2 MB).
  - output returns as per-row-int8 + f32 row scales (minimizes both the
    donated zero-init upload and the result download), decoded on host.
  - masks/identity are generated on device (affine_select).

Per-core pipeline (per batch):
  1. DMA int8 X slab rows, ACT-dequant to bf16, PE-transpose to XT
     [hid, row] (bf16).
  2. QKV projections on PE in bf16:
       QT[outd, row], KT[outd, row] (K pre-scaled on host),
       V[row, outd] (+ones col) via lhsT/rhs role swaps of XT.
  3. Attention per (512-row subpanel, head-pair): banded matmuls per
     128-row V tile rt:
       PT_raw[kv, qi] = KT-tile x QT-span   (one MM per tile, kv on psum
                                             partitions; both heads of a
                                             pair run concurrently on
                                             disjoint PE row groups)
       PT = exp(PT_raw) * mask   (ACT exp psum->bf16, DVE mask multiply)
       OT[qi, d] += PT^T x [V|1] (single PSUM accumulator; row 64 gathers
                                  the softmax denominators)
       scale rows by 1/sums into an f32 assembly buffer; per-row amax
       int8 quantization (round-to-nearest via the 1.5*2^23 trick) and
       4+4 batched DMAs (int8 data + f32 row scales) per subpanel.
"""

import sys

sys.path.insert(0, "/opt/trn_rl_repo")

import numpy as np
from concurrent.futures import ThreadPoolExecutor

B, S, HID = 2, 8192, 1024
NH, DH = 16, 64
CHUNK = 64
CORES = 8
SLICE = S // CORES          # 1024 q rows per core per batch
HALO = 128                  # 2-chunk front pad: 1 zero chunk + 1 ring-
                            # exchanged halo chunk (keeps tiles 128-aligned)
SLAB = SLICE + HALO         # 1152 rows of XT/KT per core per batch
NRT = SLAB // 128           # 9 row tiles of V / X
NSP = SLICE // 512          # 2 attention subpanels per batch
KS = 384                    # KT projection free-dim span
WSH = HID // CORES          # 128 weight rows per core shard

_CACHE = {}


def _build():
    import concourse.bass as bass
    import concourse.tile as tile
    from concourse.tile import add_dep_helper
    from concourse import mybir, bacc

    F32 = mybir.dt.float32
    BF16 = mybir.dt.bfloat16
    I8 = mybir.dt.int8
    EXP = mybir.ActivationFunctionType.Exp
    COPY = mybir.ActivationFunctionType.Copy

    nc = bacc.Bacc("TRN2", target_bir_lowering=False, debug=False,
                   num_devices=CORES)

    # all inputs ride in TWO packed blobs (one per dtype): the axon tunnel
    # charges ~60ms of round-trip latency PER ARRAY, so 7 separate inputs
    # cost ~0.3s of pure latency. Layout:
    #   bi8:  [xq (B,SLICE,HID)] [wq8 (WSH,HID)] [wk8] [wv8]
    #   bf32: [xs (B,SLICE)] [wsc (3,HID)]
    XQ_SZ = B * SLICE * HID
    WB_SZ = WSH * HID
    NI8 = XQ_SZ + 3 * WB_SZ
    NF32 = B * SLICE + 3 * HID
    bi8 = nc.dram_tensor("bi8", [NI8], I8, kind="ExternalInput")
    bf32 = nc.dram_tensor("bf32", [NF32], F32, kind="ExternalInput")

    def xq_rows(b, r0, n):  # [n, HID] int8 rows of batch b
        o = (b * SLICE + r0) * HID
        return bi8[o:o + n * HID].rearrange("(p o) -> p o", p=n)

    def xs_rows(b, r0, n):  # [n, 1] f32 row scales
        o = b * SLICE + r0
        return bf32[o:o + n].rearrange("(p o) -> p o", p=n)
    # single int8 output: 1024 data cols + 1 scale col per row. The scale
    # is linear-int8-coded as e8 = ceil((rowamax/126) * 2540) and decoded
    # host-side as e8/2540 — device quantizes with the SAME decoded value
    # (encode/decode consistent, so the coding adds only ~2% grid
    # coarsening); ceil keeps |q| <= 126. One output array instead of two
    # saves a ~60ms tunnel round trip on both the fetch and the donated
    # zero-init upload.
    out = nc.dram_tensor("out", [B, SLICE, HID + 1], I8,
                         kind="ExternalOutput")

    # qi col spans (local to a 512-col subpanel) of the band MM for V-tile
    # l = rt - 4*sp, and the PV accumulation order/splits: (l, lo, hi) with
    # lo/hi in subpanel cols; pt-tile cols are [lo - SPANS[l][0], ...).
    SPANS = [(0, 64), (0, 192), (128, 320), (256, 448), (384, 512)]
    # PV accumulation: (qi block c4, V tile l, pt col lo, pt col hi); per
    # block the full-window tile (M=128) writes first, the half-window
    # (M=64) accumulates onto partitions [0:64). All 8 MMs form one ordered
    # psum group; stop is set on the last M=128 and the last MM so the
    # per-partition group flags clear for the whole bank.
    PV_O2 = [(0, 1, 0, 128), (0, 0, 0, 64),
             (1, 2, 0, 128), (1, 1, 128, 192),
             (2, 3, 0, 128), (2, 2, 128, 192),
             (3, 4, 0, 128), (3, 3, 128, 192)]
    # mask slice of mgen [128, 192] = [D0|D1|D2] per l (mgen is generated
    # on device below: block Dd = masks for chunk-distance d / d-1)
    MSLICE = [(128, 192), (0, 192), (0, 192), (0, 192), (0, 128)]

    with tile.TileContext(nc) as tc:
        with (
            tc.tile_pool(name="dram", bufs=1, space="DRAM") as dram,
            tc.tile_pool(name="big", bufs=1) as big,
            tc.tile_pool(name="xin", bufs=4) as xin_pool,
            tc.tile_pool(name="xsc", bufs=4) as xsc_pool,
            tc.tile_pool(name="wqk", bufs=4) as wqk_pool,
            tc.tile_pool(name="wvp", bufs=2) as wv_pool,
            tc.tile_pool(name="pt", bufs=34) as pt_pool,
            tc.tile_pool(name="oacc", bufs=1) as oacc_pool,
            tc.tile_pool(name="oq", bufs=4) as oq_pool,
            tc.tile_pool(name="rec", bufs=4) as rec_pool,
            tc.tile_pool(name="misc", bufs=1) as misc,
            tc.tile_pool(name="pss", bufs=4, space="PSUM") as ps_small,
            tc.tile_pool(name="psp", bufs=2, space="PSUM") as ps_proj,
            tc.tile_pool(name="pso", bufs=2, space="PSUM") as ps_o,
        ):
            # --- weight all-gather: 1/8 int8 row shards -> full [HID, HID];
            # weights are per-hid-row int8 (amax/127), dequantized to bf16
            # at tile load with per-partition ACT scales (hid sits on
            # partitions in all three projection matmul layouts) ---
            wfull = []
            for m, name in enumerate(("wq", "wk", "wv")):
                bounce = dram.tile([WSH, HID], I8, tag=f"{name}b")
                full = dram.tile([HID, HID], I8, tag=f"{name}f")
                o = XQ_SZ + m * WB_SZ
                nc.sync.dma_start(
                    out=bounce[:],
                    in_=bi8[o:o + WB_SZ].rearrange("(p o) -> p o", p=WSH))
                nc.gpsimd.collective_compute(
                    "AllGather", mybir.AluOpType.bypass,
                    replica_groups=[list(range(CORES))],
                    ins=[bounce.opt()], outs=[full.opt()])
                wfull.append(full)
            wq_full, wk_full, wv_full = wfull
            wsc_sb = []
            for m in range(3):
                wss = misc.tile([128, 8], F32, tag=f"wsc{m}")
                o = B * SLICE + m * HID
                nc.sync.dma_start(
                    out=wss[:],
                    in_=bf32[o:o + HID].rearrange("(ht p) -> p ht", p=128))
                wsc_sb.append(wss)

            # constants generated on device (saves wire bytes): identity for
            # PE transposes and the mask family mgen = [D0|D1|D2] where
            # block Dd holds the masks for chunk-distance d (top 64 rows)
            # and d-1 (bottom 64 rows): 0 -> causal, 1 -> ones, else zeros.
            ones_sb = misc.tile([128, 128], BF16, tag="ones")
            nc.vector.memset(ones_sb[:], 1.0)
            ident_sb = misc.tile([128, 128], BF16, tag="ident")
            nc.gpsimd.affine_select(
                ident_sb[:], ones_sb[:], pattern=[[1, 128]],
                compare_op=mybir.AluOpType.is_equal, fill=0.0,
                base=0, channel_multiplier=-1)
            mgen_sb = misc.tile([128, 192], BF16, tag="mgen")
            # D0: f - p >= 0 -> top causal, bottom (p>=64) all fill(0)
            nc.gpsimd.affine_select(
                mgen_sb[:, 0:64], ones_sb[:, 0:64], pattern=[[1, 64]],
                compare_op=mybir.AluOpType.is_ge, fill=0.0,
                base=0, channel_multiplier=-1)
            # D1: f - p + 64 >= 0 -> top all ones, bottom causal
            nc.gpsimd.affine_select(
                mgen_sb[:, 64:128], ones_sb[:, 0:64], pattern=[[1, 64]],
                compare_op=mybir.AluOpType.is_ge, fill=0.0,
                base=64, channel_multiplier=-1)
            # D2: p - 64 >= 0 -> top zeros, bottom all ones
            nc.gpsimd.affine_select(
                mgen_sb[:, 128:192], ones_sb[:, 0:64], pattern=[[0, 64]],
                compare_op=mybir.AluOpType.is_ge, fill=0.0,
                base=-64, channel_multiplier=1)
            # --- halo ring-exchange: each core sends its LAST 64 rows of
            # both batches (dequantized bf16, rows on partitions: batch b at
            # partitions [64b, 64b+64)) to the next core. SPMD-safe slot
            # selection: slot j of the RS input carries the halo iff
            # j == (pid+1) mod 8 (masked multiply by an is_equal of the
            # PE-broadcast partition id); ReduceScatter(add) then hands core
            # i slot i = core (i-1)'s halo. The 64 rows of the front chunk
            # beyond the halo are masked out by mgen/mfirst anyway and are
            # fed as zeros.
            rs_in = dram.tile([CORES, 128, HID], BF16, tag="rsin")
            rs_out = dram.tile([128, HID], BF16, tag="rsout")
            hq = misc.tile([128, HID], I8, tag="hq")
            hsc = misc.tile([128, 1], F32, tag="hsc")
            hbf = misc.tile([128, HID], BF16, tag="hbf")
            for b in range(B):
                nc.sync.dma_start(out=hq[64 * b:64 * b + 64, :],
                                  in_=xq_rows(b, SLICE - 64, 64))
                nc.sync.dma_start(out=hsc[64 * b:64 * b + 64, :],
                                  in_=xs_rows(b, SLICE - 64, 64))
            nc.scalar.activation(hbf[:], hq[:], COPY, scale=hsc[:])
            pid_u = misc.tile([1, 1], mybir.dt.uint32, tag="pidu")
            nc.sync.dma_start(out=pid_u[:],
                              in_=nc.partition_id_tensor[0:1, 0:1])
            pid_b = misc.tile([1, 1], BF16, tag="pidb")
            nc.vector.tensor_copy(pid_b[:], pid_u[:])
            pidp = ps_small.tile([128, 192], F32, tag="pp", name="pidp")
            nc.tensor.matmul(pidp[:, 0:1], ones_sb[0:1, :], pid_b[:],
                             start=True, stop=True)
            pidv = misc.tile([128, 1], F32, tag="pidv")
            nc.vector.tensor_scalar_add(pidv[:], pidp[:, 0:1], 1.0)
            # mfirst = D2 slice of mgen, zeroed on core 0 (no wrap-attend
            # for the first chunk of the sequence)
            selm = misc.tile([128, 1], BF16, tag="selm")
            nc.vector.tensor_scalar(selm[:], pidv[:], 1.5, None,
                                    op0=mybir.AluOpType.is_ge)
            mfirst_sb = misc.tile([128, 64], BF16, tag="mfirst")
            nc.vector.tensor_tensor(mfirst_sb[:], mgen_sb[:, 128:192],
                                    selm[:].to_broadcast((128, 64)),
                                    mybir.AluOpType.mult)
            for j in range(CORES):
                sel = rec_pool.tile([128, 1], BF16, tag="sel", name="sel")
                nc.vector.tensor_scalar(
                    sel[:], pidv[:], float(j) if j else 8.0, None,
                    op0=mybir.AluOpType.is_equal)
                slot = oq_pool.tile([128, HID], BF16, tag="slot",
                                    name="slot")
                nc.vector.tensor_tensor(
                    slot[:], hbf[:], sel[:].to_broadcast((128, HID)),
                    mybir.AluOpType.mult)
                nc.sync.dma_start(out=rs_in[j], in_=slot[:])
            nc.gpsimd.collective_compute(
                "ReduceScatter", mybir.AluOpType.add,
                replica_groups=[list(range(CORES))],
                ins=[rs_in.opt()], outs=[rs_out.opt()])

            for b in range(B):
                XT = big.tile([128, 8, SLAB], BF16, tag="xt")
                QT = big.tile([128, 8, SLICE], BF16, tag="qt")
                KT = big.tile([128, 8, SLAB], BF16, tag="kt")
                V1 = big.tile([128, NRT, NH, DH + 1], BF16, tag="v1")
                nc.vector.memset(V1[:, :, :, DH:DH + 1], 1.0)

                # --- Phase A: load int8 + dequant + transpose X ---
                # slab tile 0 = [64 zero rows | 64 ring-exchanged halo rows];
                # tiles 1..8 come from this core's own 1024 rows.
                for rt in range(NRT):
                    xin = xin_pool.tile([128, HID], BF16, tag="xin",
                                        name="xin")
                    if rt == 0:
                        nc.vector.memset(xin[0:64, :], 0.0)
                        nc.sync.dma_start(out=xin[64:128, :],
                                          in_=rs_out[64 * b:64 * b + 64, :])
                    else:
                        r0 = 128 * rt - 128
                        xin8 = xin_pool.tile([128, HID], I8, tag="xin8",
                                             name="xin8")
                        nc.sync.dma_start(out=xin8[:],
                                          in_=xq_rows(b, r0, 128))
                        xsc = xsc_pool.tile([128, 1], F32, tag="xsc")
                        nc.sync.dma_start(out=xsc[:],
                                          in_=xs_rows(b, r0, 128))
                        nc.scalar.activation(xin[:], xin8[:], COPY,
                                             scale=xsc[:])
                    for hp in range(4):
                        # transpose passes through lhsT dtype -> bf16 psum;
                        # full-bank alloc keeps the pool slot size uniform
                        tpf = ps_proj.tile([128, 1024], BF16, tag="proj",
                                           name="tp")
                        tp = tpf[:, 0:256]
                        tm1 = nc.tensor.matmul(
                            tp[:, 0:128], xin[:, 256 * hp:256 * hp + 128],
                            ident_sb[:], is_transpose=True,
                            start=True, stop=False)
                        tm2 = nc.tensor.matmul(
                            tp[:, 128:256],
                            xin[:, 256 * hp + 128:256 * hp + 256],
                            ident_sb[:], is_transpose=True,
                            start=False, stop=True)
                        add_dep_helper(tm2.ins, tm1.ins, sync=False,
                                       reason="psum group order")
                        nc.vector.tensor_copy(
                            XT[:, 2 * hp:2 * hp + 2,
                               128 * rt:128 * rt + 128], tp[:])

                # --- Phase B: projections ---
                # QT: lhsT = wq tile [hid, outd], rhs = XT -> [outd, row] bf16
                for ot in range(8):
                    wt8 = wqk_pool.tile([128, 8, 128], I8, tag="wqk8",
                                        name="wt8")
                    nc.sync.dma_start(
                        out=wt8[:],
                        in_=wq_full[:, 128 * ot:128 * ot + 128].rearrange(
                            "(ht p) o -> p ht o", p=128))
                    wt = wqk_pool.tile([128, 8, 128], BF16, tag="wqk")
                    for ht in range(8):
                        nc.scalar.activation(wt[:, ht, :], wt8[:, ht, :],
                                             COPY,
                                             scale=wsc_sb[0][:, ht:ht + 1])
                    for half in range(2):
                        qp = ps_proj.tile([128, 512], F32, tag="proj")
                        for ht in range(8):
                            nc.tensor.matmul(
                                qp[:], wt[:, ht, :],
                                XT[:, ht, HALO + 512 * half:
                                   HALO + 512 * half + 512],
                                start=(ht == 0), stop=(ht == 7))
                        nc.vector.tensor_copy(
                            QT[:, ot, 512 * half:512 * half + 512], qp[:])

                # KT: same, over all SLAB cols (K pre-scaled on host)
                for ot in range(8):
                    wt8 = wqk_pool.tile([128, 8, 128], I8, tag="wqk8",
                                        name="wt8")
                    nc.sync.dma_start(
                        out=wt8[:],
                        in_=wk_full[:, 128 * ot:128 * ot + 128].rearrange(
                            "(ht p) o -> p ht o", p=128))
                    wt = wqk_pool.tile([128, 8, 128], BF16, tag="wqk")
                    for ht in range(8):
                        nc.scalar.activation(wt[:, ht, :], wt8[:, ht, :],
                                             COPY,
                                             scale=wsc_sb[1][:, ht:ht + 1])
                    for ks in range(SLAB // KS):
                        kpf = ps_proj.tile([128, 512], F32, tag="proj",
                                           name="kpf")
                        kp = kpf[:, 0:KS]
                        for ht in range(8):
                            nc.tensor.matmul(
                                kp[:], wt[:, ht, :],
                                XT[:, ht, KS * ks:KS * ks + KS],
                                start=(ht == 0), stop=(ht == 7))
                        nc.vector.tensor_copy(
                            KT[:, ot, KS * ks:KS * ks + KS], kp[:])

                # V: lhsT = XT row tile, rhs = wv [hid, outd] -> [row, outd]
                for oh in range(2):
                    wvt8 = wv_pool.tile([128, 8, 512], I8, tag="wv8",
                                        name="wvt8")
                    nc.sync.dma_start(
                        out=wvt8[:],
                        in_=wv_full[:, 512 * oh:512 * oh + 512].rearrange(
                            "(ht p) o -> p ht o", p=128))
                    wvt = wv_pool.tile([128, 8, 512], BF16, tag="wv")
                    for ht in range(8):
                        nc.scalar.activation(wvt[:, ht, :], wvt8[:, ht, :],
                                             COPY,
                                             scale=wsc_sb[2][:, ht:ht + 1])
                    for rt in range(NRT):
                        vp = ps_proj.tile([128, 512], F32, tag="proj")
                        for ht in range(8):
                            nc.tensor.matmul(
                                vp[:], XT[:, ht, 128 * rt:128 * rt + 128],
                                wvt[:, ht, :], start=(ht == 0),
                                stop=(ht == 7))
                        nc.vector.tensor_copy(
                            V1[:, rt, 8 * oh:8 * oh + 8, 0:DH], vp[:])

                # --- Phase C: attention ---
                for sp in range(NSP):
                    oacc = oacc_pool.tile([128, 4, HID], F32, tag="oacc")

                    def emit_mm1s(sp, t):
                        pts = {}
                        for l in (1, 0, 2, 3, 4):
                            rt = 4 * sp + l
                            lo, hi = SPANS[l]
                            pps = []
                            for e in range(2):
                                pp = ps_small.tile([128, 192], F32,
                                                   tag="pp", name="pp")
                                nc.tensor.matmul(
                                    pp[:, 0:hi - lo],
                                    KT[64 * e:64 * e + 64, t,
                                       128 * rt:128 * rt + 128],
                                    QT[64 * e:64 * e + 64, t,
                                       512 * sp + lo:512 * sp + hi],
                                    start=True, stop=True,
                                    tile_position=(64 * e, 0))
                                pps.append(pp)
                            for e in range(2):
                                pt = pt_pool.tile([128, 192], BF16, tag="pt",
                                                  name="pt")
                                nc.scalar.activation(pt[:, 0:hi - lo],
                                                     pps[e][:, 0:hi - lo],
                                                     EXP)
                                if l == 0 and sp == 0:
                                    msk = mfirst_sb[:]
                                else:
                                    ml, mh = MSLICE[l]
                                    msk = mgen_sb[:, ml:mh]
                                nc.vector.tensor_tensor(
                                    pt[:, 0:hi - lo], pt[:, 0:hi - lo], msk,
                                    mybir.AluOpType.mult)
                                pts[(e, l)] = pt
                        return pts

                    def emit_pv(sp, t, pts):
                        for e in range(2):
                            h = 2 * t + e
                            # O[qi, d] directly: lhsT = PT slice (qi block on
                            # psum partitions), rhs = [V|1]; all 4 qi blocks
                            # share one psum bank; per block the full-window
                            # tile writes first, the half-window accumulates.
                            ops = ps_o.tile([128, 4, DH + 1], F32, tag="o",
                                            name="ops")
                            prev = None
                            for i, (c4, l, plo, phi) in enumerate(PV_O2):
                                rt = 4 * sp + l
                                mm = nc.tensor.matmul(
                                    ops[0:phi - plo, c4, :],
                                    pts[(e, l)][:, plo:phi],
                                    V1[:, rt, h, :],
                                    start=(i == 0),
                                    stop=(i >= len(PV_O2) - 2),
                                    skip_group_check=True)
                                if prev is not None:
                                    # keep the per-block psum groups in
                                    # program order (flag-clear before the
                                    # next group's start)
                                    add_dep_helper(mm.ins, prev.ins,
                                                   sync=False,
                                                   reason="psum group order")
                                prev = mm
                            rec = rec_pool.tile([128, 4], F32, tag="rec")
                            nc.vector.reciprocal(rec[:], ops[:, :, DH:DH + 1])
                            nc.vector.tensor_tensor(
                                oacc[:, :, DH * h:DH * h + DH],
                                ops[:, :, 0:DH],
                                rec[:, :, None].to_broadcast((128, 4, DH)),
                                mybir.AluOpType.mult)

                    pending = []
                    for t in range(NH // 2):
                        pts = emit_mm1s(sp, t)
                        pending.append((t, pts))
                        if len(pending) > 2:
                            pt_, pts_ = pending.pop(0)
                            emit_pv(sp, pt_, pts_)
                    for pt_, pts_ in pending:
                        emit_pv(sp, pt_, pts_)

                    # int8-quantize the subpanel per out row: amax over hid,
                    # scale = amax/126 (margin vs reciprocal rounding), then
                    # round-to-nearest via the +1.5*2^23 float trick.
                    RB = 12582912.0  # 1.5 * 2**23
                    qam = rec_pool.tile([128, 4], F32, tag="qam", name="qam")
                    nc.vector.tensor_reduce(qam[:], oacc[:],
                                            mybir.AxisListType.X,
                                            mybir.AluOpType.max,
                                            apply_absolute_value=True)
                    # e8 = ceil((qam/126)*2540) via rint(x+0.5); decode
                    # scale = e8/2540 matches the device's qrec exactly
                    tq = rec_pool.tile([128, 4], F32, tag="oscs",
                                       name="oscs")
                    nc.vector.tensor_scalar(
                        tq[:], qam[:], 2540.0 / 126.0, 0.5,
                        op0=mybir.AluOpType.mult,
                        op1=mybir.AluOpType.add)
                    e8f = rec_pool.tile([128, 4], F32, tag="e8f",
                                        name="e8f")
                    nc.vector.tensor_scalar(
                        e8f[:], tq[:], RB, RB,
                        op0=mybir.AluOpType.add,
                        op1=mybir.AluOpType.subtract)
                    e8i = rec_pool.tile([128, 4], I8, tag="e8i",
                                        name="e8i")
                    nc.vector.tensor_copy(e8i[:], e8f[:])
                    scq = rec_pool.tile([128, 4], F32, tag="scq",
                                        name="scq")
                    nc.vector.tensor_scalar_mul(scq[:], e8f[:], 1.0 / 2540.0)
                    qrec = rec_pool.tile([128, 4], F32, tag="qrec",
                                         name="qrec")
                    nc.vector.reciprocal(qrec[:], scq[:])
                    for c4 in range(4):
                        r0 = 512 * sp + 128 * c4
                        t1 = oq_pool.tile([128, HID], F32, tag="t1",
                                          name="t1")
                        nc.vector.tensor_scalar(
                            t1[:], oacc[:, c4, :], qrec[:, c4:c4 + 1], RB,
                            op0=mybir.AluOpType.mult,
                            op1=mybir.AluOpType.add)
                        q8 = oq_pool.tile([128, HID], I8, tag="q8",
                                          name="q8")
                        nc.vector.tensor_scalar(
                            q8[:], t1[:], RB, None,
                            op0=mybir.AluOpType.subtract)
                        nc.sync.dma_start(out=out[b, r0:r0 + 128, 0:HID],
                                          in_=q8[:])
                        nc.sync.dma_start(out=out[b, r0:r0 + 128,
                                                  HID:HID + 1],
                                          in_=e8i[:, c4:c4 + 1])
    nc.compile()
    return nc


XQ_SZ = B * SLICE * HID
WB_SZ = WSH * HID
NI8 = XQ_SZ + 3 * WB_SZ
NF32 = B * SLICE + 3 * HID


def _pack_core(i, bi8, bf32, xq8, xsc, wq, wk, wv, wscales):
    bi8[:XQ_SZ].reshape(B, SLICE, HID)[...] = xq8[:, SLICE * i:SLICE * (i + 1)]
    for m, w in enumerate((wq, wk, wv)):
        o = XQ_SZ + m * WB_SZ
        bi8[o:o + WB_SZ].reshape(WSH, HID)[...] = w[WSH * i:WSH * (i + 1)]
    bf32[:B * SLICE].reshape(B, SLICE)[...] = xsc[:, SLICE * i:SLICE * (i + 1)]
    bf32[B * SLICE:].reshape(3, HID)[...] = wscales


def kernel(hidden_states, Wq, Wk, Wv, _trace=False):
    from concourse.bass_utils import run_bass_kernel_spmd

    hidden_states = np.asarray(hidden_states, dtype=np.float32)

    def _wquant(W, pre=1.0):
        W = np.asarray(W, dtype=np.float32) * np.float32(pre)
        am = np.maximum(np.abs(W).max(axis=1), np.float32(1e-20))  # [HID]
        sc = am * np.float32(1.0 / 127.0)
        q = np.rint(W * (np.float32(1.0) / sc)[:, None]).astype(np.int8)
        return q, sc

    Wq, sq = _wquant(Wq)
    Wk, sk = _wquant(Wk, pre=1.0 / np.sqrt(DH))
    Wv, sv = _wquant(Wv)
    wsc = np.stack([sq, sk, sv]).astype(np.float32)  # [3, HID]

    # per-row symmetric int8 quantization of hidden_states; numpy releases
    # the GIL on large array ops so chunked threads give real speedup
    if "pool" not in _CACHE:
        _CACHE["pool"] = ThreadPoolExecutor(max_workers=8)
        _CACHE["qtmp"] = np.empty((B, S, HID), dtype=np.float32)
        _CACHE["q8"] = np.empty((B, S, HID), dtype=np.int8)
    pool = _CACHE["pool"]
    tmp, xq8 = _CACHE["qtmp"], _CACHE["q8"]
    xsc = np.empty((B, S), dtype=np.float32)

    def _quant_chunk(b, s0, s1):
        h = hidden_states[b, s0:s1]
        t = tmp[b, s0:s1]
        am = np.maximum(np.maximum(h.max(axis=-1), -h.min(axis=-1)),
                        np.float32(1e-20))
        xsc[b, s0:s1] = am * np.float32(1.0 / 127.0)
        np.multiply(h, (np.float32(127.0) / am)[:, None], out=t)
        np.rint(t, out=t)
        xq8[b, s0:s1] = t

    CH = S // 4
    list(pool.map(lambda a: _quant_chunk(*a),
                  [(b, c * CH, (c + 1) * CH) for b in range(B)
                   for c in range(4)]))

    if "nc" not in _CACHE:
        _CACHE["nc"] = _build()
    nc = _CACHE["nc"]

    if "bi8" not in _CACHE:
        _CACHE["bi8"] = np.empty((CORES, NI8), dtype=np.int8)
        _CACHE["bf32"] = np.empty((CORES, NF32), dtype=np.float32)
    bi8_all, bf32_all = _CACHE["bi8"], _CACHE["bf32"]
    list(_CACHE["pool"].map(
        lambda i: _pack_core(i, bi8_all[i], bf32_all[i], xq8, xsc,
                             Wq, Wk, Wv, wsc), range(CORES)))
    in_maps = [{"bi8": bi8_all[i], "bf32": bf32_all[i]}
               for i in range(CORES)]
    res = run_bass_kernel_spmd(nc, in_maps, list(range(CORES)), trace=_trace)
    _CACHE["last"] = res
    full = np.empty((B, S, HID), dtype=np.float32)

    def _decode(i):
        raw = res.results[i]["out"]
        sc = raw[:, :, HID:].astype(np.float32) * np.float32(1.0 / 2540.0)
        np.multiply(raw[:, :, :HID], sc,
                    out=full[:, SLICE * i:SLICE * (i + 1), :])

    list(_CACHE["pool"].map(_decode, range(CORES)))
    return full
